# revision 1
# baseline (speedup 1.0000x reference)
"""Trainium2 Bass kernel for nn_Attention_xxc (dense transformer attention
with hop-distance bias). Data-parallel over batch: 8 cores x 2 batches.

Layout strategy (per core):
  - Host preps transposed inputs: xT [512, 2048], WqkvT [512, 1536] (q cols
    pre-scaled by 1/sqrt(hd)), WprojT [512, 512], biasT[h] = (alpha_h *
    sum_k w_hk Hstack_k).T in bf16.
  - qkv: q,k computed TRANSPOSED ([outch, tok], bf16), v computed NATURAL
    ([tok, vch], bf16) with a ones-column appended per head (65 cols/head).
  - scores computed transposed: S.T[m, n] = k_m . q_n + bias.T  (bias folded
    in via identity-matmul PSUM accumulation), exp on ACT -> P bf16.
  - AV: out_aug.T[d(+1), n] = v_aug.T @ P ; row 64 = softmax denominator.
  - normalize: broadcast 1/denom across partitions via K=1 matmul, multiply.
  - proj: y[n, o] = outT.T @ WprojT + bproj, natural layout, DMA out.
"""
import sys

sys.path.insert(0, "/opt/trn_rl_repo")

import numpy as np
import ml_dtypes

B, N, DIM = 16, 1024, 512
H, HD, KH = 8, 64, 5
SCALE = HD ** -0.5
NCORES = 8
BPC = B // NCORES          # batches per core
TOK = BPC * N              # tokens per core = 2048

_CACHE = {}


def _build():
    import concourse.bass as bass
    import concourse.bacc as bacc
    import concourse.mybir as mybir
    from concourse.tile import TileContext

    f32 = mybir.dt.float32
    f32r = mybir.dt.float32r
    bf16 = mybir.dt.bfloat16
    EXP = mybir.ActivationFunctionType.Exp
    CPY = mybir.ActivationFunctionType.Copy
    MUL = mybir.AluOpType.mult
    ADD = mybir.AluOpType.add

    nc = bacc.Bacc()
    xT = nc.declare_dram_parameter("xT", [DIM, TOK], bf16, isOutput=False)
    wqkvT = nc.declare_dram_parameter("wqkvT", [DIM, 3 * DIM], bf16, isOutput=False)
    wprojT = nc.declare_dram_parameter("wprojT", [DIM, DIM], bf16, isOutput=False)
    bprojb = nc.declare_dram_parameter("bprojb", [128, DIM], f32, isOutput=False)
    biasT = nc.declare_dram_parameter("biasT", [H, N, N], bf16, isOutput=False)
    eye = nc.declare_dram_parameter("eye", [128, 128], bf16, isOutput=False)
    ones64 = nc.declare_dram_parameter("ones64", [1, 64], bf16, isOutput=False)
    y = nc.declare_dram_parameter("y", [TOK, DIM], f32, isOutput=True)

    NT = TOK // 128            # 16 token tiles
    VW = H * (HD + 1)          # 520: v row width with ones col per head

    with TileContext(nc) as tc:
        with (
            tc.tile_pool(name="qk", bufs=1) as QK,
            tc.tile_pool(name="vres", bufs=1) as VR,
            tc.tile_pool(name="wp", bufs=1) as WP,
            tc.tile_pool(name="outT", bufs=1) as OT,
            tc.tile_pool(name="const", bufs=1) as CONST,
        ):
            eye_t = CONST.tile([128, 128], bf16, tag="eye", name="eye")
            nc.sync.dma_start(out=eye_t[:], in_=eye[:])
            ones_t = CONST.tile([1, 64], bf16, tag="ones", name="ones")
            nc.sync.dma_start(out=ones_t[:], in_=ones64[:])
            bpb_t = CONST.tile([128, DIM], f32, tag="bpb", name="bpb")
            nc.sync.dma_start(out=bpb_t[:], in_=bprojb[:])
            wp_t = [WP.tile([128, DIM], bf16, tag=f"wp{c}", name=f"wp{c}") for c in range(4)]
            for c in range(4):
                nc.sync.dma_start(out=wp_t[c][:], in_=wprojT[c * 128:(c + 1) * 128, :])

            qk_t = [QK.tile([128, TOK], bf16, tag=f"qk{o}", name=f"qk{o}") for o in range(8)]
            v_t = [VR.tile([128, VW], bf16, tag=f"v{t}", name=f"v{t}") for t in range(NT)]
            oT_t = [OT.tile([128, N], bf16, tag=f"oT{b}_{c}", name=f"oT{b}_{c}")
                    for b in range(BPC) for c in range(4)]

            # ---------------- phase 1: qkv projections ----------------
            with (
                tc.tile_pool(name="xw", bufs=1) as XW,
                tc.tile_pool(name="ps1", bufs=4, space="PSUM") as PS1,
            ):
                xT_t = [XW.tile([128, TOK], bf16, tag=f"x{c}", name=f"x{c}") for c in range(4)]
                wq_t = [XW.tile([128, 3 * DIM], bf16, tag=f"w{c}", name=f"w{c}") for c in range(4)]
                for c in range(4):
                    nc.sync.dma_start(out=xT_t[c][:], in_=xT[c * 128:(c + 1) * 128, :])
                    nc.sync.dma_start(out=wq_t[c][:], in_=wqkvT[c * 128:(c + 1) * 128, :])

                # q,k transposed: qkvT[o_tile, tok] ; o tiles 0..7 cover q,k
                for o in range(8):
                    for t in range(4):           # tok chunks of 512
                        ps = PS1.tile([128, 512], f32, tag="ps1", name="ps1")
                        for c in range(4):
                            nc.tensor.matmul(
                                ps[:], wq_t[c][:, o * 128:(o + 1) * 128],
                                xT_t[c][:, t * 512:(t + 1) * 512],
                                start=(c == 0), stop=(c == 3))
                        nc.vector.tensor_copy(qk_t[o][:, t * 512:(t + 1) * 512], ps[:])
                # v natural: [tok_tile, vch] -> packed per head with ones col
                for t in range(NT):
                    ps = PS1.tile([128, 512], f32, tag="ps1", name="ps1")
                    for c in range(4):
                        nc.tensor.matmul(
                            ps[:], xT_t[c][:, t * 128:(t + 1) * 128],
                            wq_t[c][:, 2 * DIM:3 * DIM],
                            start=(c == 0), stop=(c == 3))
                    dst = v_t[t][:, 0:VW].rearrange("p (h s) -> p h s", s=HD + 1)
                    nc.vector.tensor_copy(
                        dst[:, :, 0:HD],
                        ps[:].rearrange("p (h s) -> p h s", s=HD))
                    nc.vector.memset(dst[:, :, HD:HD + 1], 1.0)

            # ---------------- phase 2: attention ----------------
            with (
                tc.tile_pool(name="biasp", bufs=18) as BP,
                tc.tile_pool(name="pp", bufs=14) as PP,
                tc.tile_pool(name="nrm", bufs=4) as NRM,
                tc.tile_pool(name="ysb", bufs=3) as YSB,
                tc.tile_pool(name="pss", bufs=2, space="PSUM") as PSS,
                tc.tile_pool(name="pso", bufs=1, space="PSUM") as PSO,
                tc.tile_pool(name="psm", bufs=2, space="PSUM") as PSM,
            ):
                for h in range(H):
                    qt, po = qk_t[h // 2], (h % 2) * 64
                    kt = qk_t[4 + h // 2]
                    b_tiles = []
                    for mi in range(8):
                        bt = BP.tile([128, N], bf16, tag="bias", name="bias")
                        nc.sync.dma_start(
                            out=bt[:], in_=biasT[h, mi * 128:(mi + 1) * 128, :])
                        b_tiles.append(bt)
                    for b in range(BPC):
                        t0 = b * N
                        p_tiles = []
                        for mi in range(8):
                            ps = PSS.tile([128, N], f32, tag="pss", name="pss")
                            for nchunk in range(2):
                                sl = slice(nchunk * 512, (nchunk + 1) * 512)
                                nc.tensor.matmul(
                                    ps[:, sl],
                                    kt[po:po + 64, t0 + mi * 128: t0 + (mi + 1) * 128],
                                    qt[po:po + 64, t0 + nchunk * 512: t0 + (nchunk + 1) * 512],
                                    start=True, stop=False)
                                nc.tensor.matmul(
                                    ps[:, sl], eye_t[:], b_tiles[mi][:, sl],
                                    start=False, stop=True)
                            pt = PP.tile([128, N], bf16, tag="p", name="p")
                            nc.scalar.activation(pt[:], ps[:], EXP)
                            p_tiles.append(pt)
                        pso = PSO.tile([HD + 1, N], f32, tag="pso", name="pso")
                        for mi in range(8):
                            for nchunk in range(2):
                                sl = slice(nchunk * 512, (nchunk + 1) * 512)
                                nc.tensor.matmul(
                                    pso[:, sl],
                                    v_t[b * 8 + mi][:, h * (HD + 1):(h + 1) * (HD + 1)],
                                    p_tiles[mi][:, sl],
                                    start=(mi == 0), stop=(mi == 7))
                        # denominator -> broadcast -> reciprocal -> normalize
                        d_t = NRM.tile([1, N], bf16, tag="d", name="d")
                        nc.vector.tensor_copy(d_t[:], pso[64:65, :])
                        R_t = NRM.tile([64, N], f32, tag="R", name="R")
                        for nchunk in range(2):
                            sl = slice(nchunk * 512, (nchunk + 1) * 512)
                            psr = PSM.tile([64, 512], f32, tag="psm", name="psm")
                            nc.tensor.matmul(psr[:], ones_t[:], d_t[:, sl],
                                             start=True, stop=True)
                            nc.vector.reciprocal(R_t[:, sl], psr[:])
                        nc.vector.tensor_tensor(
                            oT_t[b * 4 + h // 2][po:po + 64, :],
                            pso[0:64, :], R_t[:], MUL)
                # ---------------- phase 3: output projection ----------------
                for b in range(BPC):
                    for t in range(8):
                        psy = PSM.tile([128, 512], f32, tag="psm", name="psm")
                        for c in range(4):
                            nc.tensor.matmul(
                                psy[:],
                                oT_t[b * 4 + c][:, t * 128:(t + 1) * 128],
                                wp_t[c][:], start=(c == 0), stop=(c == 3))
                        yt = YSB.tile([128, DIM], f32, tag="y", name="y")
                        nc.vector.tensor_tensor(yt[:], psy[:], bpb_t[:], ADD)
                        nc.sync.dma_start(
                            out=y[b * N + t * 128: b * N + (t + 1) * 128, :],
                            in_=yt[:])
    nc.compile()
    return nc


def _prep_host(x, Hstack, hop_logits_attn, rel_alpha, Wqkv, Wproj, bproj):
    bf = ml_dtypes.bfloat16
    lg = hop_logits_attn - hop_logits_attn.max(-1, keepdims=True)
    w = np.exp(lg)
    w /= w.sum(-1, keepdims=True)                      # [H, KH]
    Bh = np.einsum("hk,kij->hij", w.astype(np.float32),
                   Hstack.astype(np.float32))          # [H, N, N]
    biasT = np.ascontiguousarray(
        (rel_alpha[:, None, None] * Bh).transpose(0, 2, 1)).astype(bf)
    wqkvT = np.ascontiguousarray(Wqkv.T).astype(np.float32).copy()
    wqkvT[:, :DIM] *= SCALE                            # fold q scaling
    wqkvT = wqkvT.astype(bf)
    wprojT = np.ascontiguousarray(Wproj.T).astype(bf)
    bprojb = np.tile(bproj[None, :], (128, 1)).astype(np.float32)
    eye = np.eye(128, dtype=np.float32).astype(bf)
    ones64 = np.ones((1, 64), dtype=np.float32).astype(bf)
    shared = dict(wqkvT=wqkvT, wprojT=wprojT, bprojb=bprojb,
                  biasT=biasT, eye=eye, ones64=ones64)
    in_maps = []
    for i in range(NCORES):
        xi = x[i * BPC:(i + 1) * BPC].reshape(TOK, DIM)
        xTi = np.ascontiguousarray(xi.T).astype(bf)
        in_maps.append(dict(xT=xTi, **shared))
    return in_maps


def kernel(**inputs):
    from concourse.bass_utils import run_bass_kernel_spmd

    if "nc" not in _CACHE:
        _CACHE["nc"] = _build()
    nc = _CACHE["nc"]
    in_maps = _prep_host(
        np.asarray(inputs["x"], np.float32),
        np.asarray(inputs["Hstack"], np.float32),
        np.asarray(inputs["hop_logits_attn"], np.float32),
        np.asarray(inputs["rel_alpha"], np.float32),
        np.asarray(inputs["Wqkv"], np.float32),
        np.asarray(inputs["Wproj"], np.float32),
        np.asarray(inputs["bproj"], np.float32))
    res = run_bass_kernel_spmd(nc, in_maps, list(range(NCORES))).results
    out = np.concatenate([r["y"].reshape(BPC, N, DIM) for r in res], axis=0)
    return out.astype(np.float32)



# revision 2
# speedup vs baseline: 3.5662x; 3.5662x over previous
"""Trainium2 Bass kernel for nn_Attention_xxc (dense transformer attention
with hop-distance bias). Data-parallel over batch: 8 cores x 2 batches.

Transfer-optimized v2: the metric is warm wall time of
run_bass_kernel_spmd, which is dominated by host->device transfer over the
axon tunnel (~13ns/B in, ~27ns/B out), not device compute (~1ms). So:
  - ONE packed input tensor per core [1889, 1024] bf16 (~3.9MB vs 20.3MB):
    xT + 1/8 shard of HstackT + 1/8 shard of [Wqkv.T|Wproj.T] + 1/8 shard
    of the per-(head,hop) scaled-identity blocks + eye/ones/bproj row.
  - Device AllGathers rebuild full HstackT / weights / c_eye from shards
    (intra-chip interconnect instead of 8x replicated host transfer).
  - Bias pre-phase computes biasT[h] = sum_k (alpha_h w_hk) HstackT_k via
    scaled-identity matmuls into a DRAM scratch; attention phase then
    matches the proven v1 structure (bias folded into scores via
    identity-matmul PSUM accumulation).
  - Output y in fp16 (halves output bytes; zeros-donation makes output
    bytes cost ~2x input bytes).

Per-core layout (as v1): q,k transposed bf16; v natural with ones column
per head (denominator); S.T = k.q + bias.T; exp on ACT; AV -> out.T with
row 64 = denom; normalize via reciprocal broadcast; proj + bproj -> y.
"""
import sys

sys.path.insert(0, "/opt/trn_rl_repo")

import numpy as np
import ml_dtypes

B, N, DIM = 16, 1024, 512
H, HD, KH = 8, 64, 5
SCALE = HD ** -0.5
NCORES = 8
BPC = B // NCORES          # batches per core
TOK = BPC * N              # tokens per core = 2048

# packed input row offsets (1024-wide bf16 rows)
ROW_X = 0        # 1024 rows: xT [512, 2048]
ROW_H = 1024     # 640 rows: HstackT shard [640, 1024]
ROW_W = 1664     # 128 rows: weight shard [64, 2048]
ROW_CE = 1792    # 80 rows: c_eye shard [16, 5120]
ROW_EYE = 1872   # 16 rows: eye [128, 128]
ROW_MISC = 1888  # 1 row: ones [0:128], bproj [128:640]
PACK_ROWS = 1889

_CACHE = {}


def _build():
    import concourse.bass as bass
    import concourse.bacc as bacc
    import concourse.mybir as mybir
    from concourse.tile import TileContext

    f32 = mybir.dt.float32
    bf16 = mybir.dt.bfloat16
    f16 = mybir.dt.float16
    EXP = mybir.ActivationFunctionType.Exp
    MUL = mybir.AluOpType.mult
    ADD = mybir.AluOpType.add
    BYP = mybir.AluOpType.bypass

    nc = bacc.Bacc(num_devices=NCORES)
    pack = nc.declare_dram_parameter("pack", [PACK_ROWS, 1024], bf16, isOutput=False)
    y = nc.declare_dram_parameter("y", [TOK, DIM], f16, isOutput=True)

    NT = TOK // 128            # 16 token tiles
    VW = H * (HD + 1)          # 520: v row width with ones col per head
    RG = [list(range(NCORES))]

    with TileContext(nc) as tc:
        with (
            tc.tile_pool(name="dram", bufs=1, space="DRAM") as DR,
            tc.tile_pool(name="const", bufs=1) as CONST,
            tc.tile_pool(name="wp", bufs=1) as WP,
            tc.tile_pool(name="qk", bufs=1) as QK,
            tc.tile_pool(name="vres", bufs=1) as VR,
            tc.tile_pool(name="outT", bufs=1) as OT,
        ):
            # ---- DRAM bounces, AllGathers (overlap with qkv phase) ----
            hb = DR.tile([640, 1024], bf16, tag="hb", name="hb")
            agh = DR.tile([5 * 1024, 1024], bf16, tag="agh", name="agh")
            wb = DR.tile([128, 1024], bf16, tag="wb", name="wb")
            agw = DR.tile([1024, 1024], bf16, tag="agw", name="agw")
            cb = DR.tile([80, 1024], bf16, tag="cb", name="cb")
            agc = DR.tile([640, 1024], bf16, tag="agc", name="agc")
            btd = DR.tile([H * 1024, 1024], bf16, tag="btd", name="btd")

            nc.sync.dma_start(out=hb[:], in_=pack[ROW_H:ROW_H + 640, :])
            nc.sync.dma_start(out=wb[:], in_=pack[ROW_W:ROW_W + 128, :])
            nc.sync.dma_start(out=cb[:], in_=pack[ROW_CE:ROW_CE + 80, :])
            nc.gpsimd.collective_compute(
                "AllGather", BYP, RG, [hb[:].opt()], [agh[:].opt()])
            nc.gpsimd.collective_compute(
                "AllGather", BYP, RG, [wb[:].opt()], [agw[:].opt()])
            nc.gpsimd.collective_compute(
                "AllGather", BYP, RG, [cb[:].opt()], [agc[:].opt()])

            # ---- constants from the pack ----
            eye_t = CONST.tile([128, 128], bf16, tag="eye", name="eye")
            nc.sync.dma_start(
                out=eye_t[:],
                in_=pack[ROW_EYE:ROW_EYE + 16, :].rearrange(
                    "a (b c) -> (a b) c", b=8))
            ones_t = CONST.tile([1, 128], bf16, tag="ones", name="ones")
            nc.sync.dma_start(out=ones_t[:], in_=pack[ROW_MISC:ROW_MISC + 1, 0:128])
            bpr_t = CONST.tile([1, DIM], bf16, tag="bpr", name="bpr")
            nc.sync.dma_start(out=bpr_t[:], in_=pack[ROW_MISC:ROW_MISC + 1, 128:640])
            ce_t = CONST.tile([128, H * KH * 128], bf16, tag="ce", name="ce")
            nc.sync.dma_start(
                out=ce_t[:], in_=agc[:].rearrange("(p r) c -> p (r c)", r=5))

            # weights from the gathered pack [512, 2048] = [wqkvT | wprojT]
            wq_t = [WP.tile([128, 3 * DIM], bf16, tag=f"wq{c}", name=f"wq{c}")
                    for c in range(4)]
            wp_t = [WP.tile([128, DIM], bf16, tag=f"wp{c}", name=f"wp{c}")
                    for c in range(4)]
            for c in range(4):
                src = agw[c * 256:(c + 1) * 256, :].rearrange(
                    "(p r) c -> p (r c)", r=2)
                nc.sync.dma_start(out=wq_t[c][:], in_=src[:, 0:3 * DIM])
                nc.sync.dma_start(out=wp_t[c][:], in_=src[:, 3 * DIM:4 * DIM])

            qk_t = [QK.tile([128, TOK], bf16, tag=f"qk{o}", name=f"qk{o}")
                    for o in range(8)]
            v_t = [VR.tile([128, VW], bf16, tag=f"v{t}", name=f"v{t}")
                   for t in range(NT)]
            oT_t = [OT.tile([128, N], bf16, tag=f"oT{b}_{c}", name=f"oT{b}_{c}")
                    for b in range(BPC) for c in range(4)]
            bpb_t = CONST.tile([128, DIM], f32, tag="bpb", name="bpb")

            # ---------------- phase 1: qkv projections ----------------
            with (
                tc.tile_pool(name="xw", bufs=1) as XW,
                tc.tile_pool(name="ps1", bufs=4, space="PSUM") as PS1,
            ):
                # bproj broadcast to 128 partitions via outer product
                psb0 = PS1.tile([128, DIM], f32, tag="ps1", name="ps1")
                nc.tensor.matmul(psb0[:], ones_t[:], bpr_t[:],
                                 start=True, stop=True)
                nc.vector.tensor_copy(bpb_t[:], psb0[:])

                xT_t = [XW.tile([128, TOK], bf16, tag=f"x{c}", name=f"x{c}")
                        for c in range(4)]
                for c in range(4):
                    nc.sync.dma_start(
                        out=xT_t[c][:],
                        in_=pack[c * 256:(c + 1) * 256, :].rearrange(
                            "(p r) c -> p (r c)", r=2))

                # q,k transposed: qkvT[o_tile, tok] ; o tiles 0..7 cover q,k
                for o in range(8):
                    for t in range(4):           # tok chunks of 512
                        ps = PS1.tile([128, 512], f32, tag="ps1", name="ps1")
                        for c in range(4):
                            nc.tensor.matmul(
                                ps[:], wq_t[c][:, o * 128:(o + 1) * 128],
                                xT_t[c][:, t * 512:(t + 1) * 512],
                                start=(c == 0), stop=(c == 3))
                        nc.vector.tensor_copy(qk_t[o][:, t * 512:(t + 1) * 512], ps[:])
                # v natural: [tok_tile, vch] -> packed per head with ones col
                for t in range(NT):
                    ps = PS1.tile([128, 512], f32, tag="ps1", name="ps1")
                    for c in range(4):
                        nc.tensor.matmul(
                            ps[:], xT_t[c][:, t * 128:(t + 1) * 128],
                            wq_t[c][:, 2 * DIM:3 * DIM],
                            start=(c == 0), stop=(c == 3))
                    dst = v_t[t][:, 0:VW].rearrange("p (h s) -> p h s", s=HD + 1)
                    nc.vector.tensor_copy(
                        dst[:, :, 0:HD],
                        ps[:].rearrange("p (h s) -> p h s", s=HD))
                    nc.vector.memset(dst[:, :, HD:HD + 1], 1.0)

            # ------- phase 1.5: bias mixture -> biasT DRAM scratch -------
            # biasT[h][m,n] = sum_k c_hk * HstackT_k[m,n], via scaled-eye
            # stationaries ce_t[:, (h*5+k)*128 : +128].
            with (
                tc.tile_pool(name="hbt", bufs=10) as HBT,
                tc.tile_pool(name="bw", bufs=4) as BW,
                tc.tile_pool(name="psb", bufs=4, space="PSUM") as PSB,
            ):
                for mi in range(8):
                    h_tiles = []
                    for k in range(KH):
                        ht = HBT.tile([128, N], bf16, tag="ht", name="ht")
                        nc.sync.dma_start(
                            out=ht[:],
                            in_=agh[k * 1024 + mi * 128:
                                    k * 1024 + (mi + 1) * 128, :])
                        h_tiles.append(ht)
                    for h in range(H):
                        psb = PSB.tile([128, N], f32, tag="psb", name="psb")
                        for chunk in range(2):
                            sl = slice(chunk * 512, (chunk + 1) * 512)
                            for k in range(KH):
                                idx = h * KH + k
                                nc.tensor.matmul(
                                    psb[:, sl],
                                    ce_t[:, idx * 128:(idx + 1) * 128],
                                    h_tiles[k][:, sl],
                                    start=(k == 0), stop=(k == KH - 1))
                        bt = BW.tile([128, N], bf16, tag="bt", name="bt")
                        nc.vector.tensor_copy(bt[:], psb[:])
                        nc.sync.dma_start(
                            out=btd[h * 1024 + mi * 128:
                                    h * 1024 + (mi + 1) * 128, :],
                            in_=bt[:])

            # ---------------- phase 2: attention ----------------
            with (
                tc.tile_pool(name="biasp", bufs=18) as BP,
                tc.tile_pool(name="pp", bufs=14) as PP,
                tc.tile_pool(name="nrm", bufs=4) as NRM,
                tc.tile_pool(name="ysb", bufs=3) as YSB,
                tc.tile_pool(name="pss", bufs=2, space="PSUM") as PSS,
                tc.tile_pool(name="pso", bufs=1, space="PSUM") as PSO,
                tc.tile_pool(name="psm", bufs=2, space="PSUM") as PSM,
            ):
                for h in range(H):
                    qt, po = qk_t[h // 2], (h % 2) * 64
                    kt = qk_t[4 + h // 2]
                    b_tiles = []
                    for mi in range(8):
                        bt = BP.tile([128, N], bf16, tag="bias", name="bias")
                        nc.sync.dma_start(
                            out=bt[:],
                            in_=btd[h * 1024 + mi * 128:
                                    h * 1024 + (mi + 1) * 128, :])
                        b_tiles.append(bt)
                    for b in range(BPC):
                        t0 = b * N
                        p_tiles = []
                        for mi in range(8):
                            ps = PSS.tile([128, N], f32, tag="pss", name="pss")
                            for nchunk in range(2):
                                sl = slice(nchunk * 512, (nchunk + 1) * 512)
                                nc.tensor.matmul(
                                    ps[:, sl],
                                    kt[po:po + 64, t0 + mi * 128: t0 + (mi + 1) * 128],
                                    qt[po:po + 64, t0 + nchunk * 512: t0 + (nchunk + 1) * 512],
                                    start=True, stop=False)
                                nc.tensor.matmul(
                                    ps[:, sl], eye_t[:], b_tiles[mi][:, sl],
                                    start=False, stop=True)
                            pt = PP.tile([128, N], bf16, tag="p", name="p")
                            nc.scalar.activation(pt[:], ps[:], EXP)
                            p_tiles.append(pt)
                        pso = PSO.tile([HD + 1, N], f32, tag="pso", name="pso")
                        for mi in range(8):
                            for nchunk in range(2):
                                sl = slice(nchunk * 512, (nchunk + 1) * 512)
                                nc.tensor.matmul(
                                    pso[:, sl],
                                    v_t[b * 8 + mi][:, h * (HD + 1):(h + 1) * (HD + 1)],
                                    p_tiles[mi][:, sl],
                                    start=(mi == 0), stop=(mi == 7))
                        # denominator -> broadcast -> reciprocal -> normalize
                        d_t = NRM.tile([1, N], bf16, tag="d", name="d")
                        nc.vector.tensor_copy(d_t[:], pso[64:65, :])
                        R_t = NRM.tile([64, N], f32, tag="R", name="R")
                        for nchunk in range(2):
                            sl = slice(nchunk * 512, (nchunk + 1) * 512)
                            psr = PSM.tile([64, 512], f32, tag="psm", name="psm")
                            nc.tensor.matmul(psr[:], ones_t[:, 0:64], d_t[:, sl],
                                             start=True, stop=True)
                            nc.vector.reciprocal(R_t[:, sl], psr[:])
                        nc.vector.tensor_tensor(
                            oT_t[b * 4 + h // 2][po:po + 64, :],
                            pso[0:64, :], R_t[:], MUL)
                # ---------------- phase 3: output projection ----------------
                for b in range(BPC):
                    for t in range(8):
                        psy = PSM.tile([128, 512], f32, tag="psm", name="psm")
                        for c in range(4):
                            nc.tensor.matmul(
                                psy[:],
                                oT_t[b * 4 + c][:, t * 128:(t + 1) * 128],
                                wp_t[c][:], start=(c == 0), stop=(c == 3))
                        yt = YSB.tile([128, DIM], f16, tag="y", name="y")
                        nc.vector.tensor_tensor(yt[:], psy[:], bpb_t[:], ADD)
                        nc.sync.dma_start(
                            out=y[b * N + t * 128: b * N + (t + 1) * 128, :],
                            in_=yt[:])
    nc.compile()
    return nc


def _prep_host(x, Hstack, hop_logits_attn, rel_alpha, Wqkv, Wproj, bproj):
    bf = ml_dtypes.bfloat16
    lg = hop_logits_attn - hop_logits_attn.max(-1, keepdims=True)
    w = np.exp(lg)
    w /= w.sum(-1, keepdims=True)                      # [H, KH]
    c = (rel_alpha[:, None] * w).astype(np.float32)    # [H, KH]
    # scaled-identity blocks [128, H*KH*128]
    ce = np.zeros((128, H * KH * 128), np.float32)
    eye128 = np.eye(128, dtype=np.float32)
    for h in range(H):
        for k in range(KH):
            idx = h * KH + k
            ce[:, idx * 128:(idx + 1) * 128] = c[h, k] * eye128
    ce = ce.astype(bf)

    HstackT = np.ascontiguousarray(
        Hstack.astype(np.float32).transpose(0, 2, 1)).reshape(5120, 1024).astype(bf)
    wqkvT = np.ascontiguousarray(Wqkv.T).astype(np.float32).copy()
    wqkvT[:, :DIM] *= SCALE                            # fold q scaling
    wpack = np.concatenate(
        [wqkvT, np.ascontiguousarray(Wproj.T)], axis=1).astype(bf)  # [512, 2048]
    eye_bf = eye128.astype(bf)

    in_maps = []
    for i in range(NCORES):
        pk = np.zeros((PACK_ROWS, 1024), bf)
        xi = x[i * BPC:(i + 1) * BPC].reshape(TOK, DIM)
        pk[ROW_X:ROW_X + 1024] = np.ascontiguousarray(xi.T).astype(bf).reshape(1024, 1024)
        pk[ROW_H:ROW_H + 640] = HstackT[i * 640:(i + 1) * 640]
        pk[ROW_W:ROW_W + 128] = wpack[i * 64:(i + 1) * 64].reshape(128, 1024)
        pk[ROW_CE:ROW_CE + 80] = ce[i * 16:(i + 1) * 16].reshape(80, 1024)
        pk[ROW_EYE:ROW_EYE + 16] = eye_bf.reshape(16, 1024)
        pk[ROW_MISC, 0:128] = 1.0
        pk[ROW_MISC, 128:640] = bproj.astype(bf)
        in_maps.append(dict(pack=pk))
    return in_maps


def kernel(**inputs):
    from concourse.bass_utils import run_bass_kernel_spmd

    if "nc" not in _CACHE:
        _CACHE["nc"] = _build()
    nc = _CACHE["nc"]
    in_maps = _prep_host(
        np.asarray(inputs["x"], np.float32),
        np.asarray(inputs["Hstack"], np.float32),
        np.asarray(inputs["hop_logits_attn"], np.float32),
        np.asarray(inputs["rel_alpha"], np.float32),
        np.asarray(inputs["Wqkv"], np.float32),
        np.asarray(inputs["Wproj"], np.float32),
        np.asarray(inputs["bproj"], np.float32))
    res = run_bass_kernel_spmd(nc, in_maps, list(range(NCORES))).results
    out = np.concatenate(
        [r["y"].astype(np.float32).reshape(BPC, N, DIM) for r in res], axis=0)
    return out


# revision 8
# speedup vs baseline: 3.6719x; 1.0296x over previous
"""Trainium2 Bass kernel for nn_Attention_xxc (dense transformer attention
with hop-distance bias). Data-parallel over batch: 8 cores x 2 batches.

Transfer-optimized v3: the metric is warm wall time of
run_bass_kernel_spmd, which is dominated by host->device transfer over the
axon tunnel (~13ns/B in, ~27ns/B out), not device compute (~1ms). So:
  - ONE packed input tensor per core [1889, 1024] bf16 (~3.9MB vs 20.3MB):
    xT + HstackT m-row shard [5,128,1024] + 1/8 shard of [Wqkv.T|Wproj.T]
    + 1/8 shard of the per-(head,hop) scaled-identity blocks +
    eye/ones/bproj row.
  - Bias work is sharded by m-rows: core c computes
    biasT[h][128c:128c+128, :] = sum_k (alpha_h w_hk) HstackT_k[rows] for
    ALL heads from its own pack (scaled-identity matmuls), then ONE
    AllGather distributes the full 16MB biasT. Weights and c_eye shards
    are AllGathered too (intra-chip interconnect instead of 8x replicated
    host transfer). Attention phase matches the proven v1 structure (bias
    folded into scores via identity-matmul PSUM accumulation).
  - Output y in fp16 (halves output bytes; zeros-donation makes output
    bytes cost ~2x input bytes).

Per-core layout (as v1): q,k transposed bf16; v natural with ones column
per head (denominator); S.T = k.q + bias.T; exp on ACT; AV -> out.T with
row 64 = denom; normalize via reciprocal broadcast; proj + bproj -> y.
"""
import sys

sys.path.insert(0, "/opt/trn_rl_repo")

import numpy as np
import ml_dtypes

B, N, DIM = 16, 1024, 512
H, HD, KH = 8, 64, 5
SCALE = HD ** -0.5
NCORES = 8
BPC = B // NCORES          # batches per core
TOK = BPC * N              # tokens per core = 2048

# packed input row offsets (1024-wide bf16 rows)
ROW_X = 0        # 1024 rows: xT [512, 2048]
ROW_H = 1024     # 640 rows: HstackT shard [640, 1024]
ROW_W = 1664     # 128 rows: weight shard [64, 2048]
ROW_CE = 1792    # 80 rows: c_eye shard [16, 5120]
ROW_EYE = 1872   # 16 rows: eye [128, 128]
ROW_MISC = 1888  # 1 row: ones [0:128], bproj [128:640]
PACK_ROWS = 1889

_CACHE = {}


def _build():
    import concourse.bass as bass
    import concourse.bacc as bacc
    import concourse.mybir as mybir
    from concourse.tile import TileContext

    f32 = mybir.dt.float32
    bf16 = mybir.dt.bfloat16
    f16 = mybir.dt.float16
    EXP = mybir.ActivationFunctionType.Exp
    MUL = mybir.AluOpType.mult
    ADD = mybir.AluOpType.add
    BYP = mybir.AluOpType.bypass

    nc = bacc.Bacc(num_devices=NCORES)
    pack = nc.declare_dram_parameter("pack", [PACK_ROWS, 1024], bf16, isOutput=False)
    y = nc.declare_dram_parameter("y", [TOK, DIM], f16, isOutput=True)

    NT = TOK // 128            # 16 token tiles
    VW = H * (HD + 1)          # 520: v row width with ones col per head
    RG = [list(range(NCORES))]

    with TileContext(nc) as tc:
        with (
            tc.tile_pool(name="dram", bufs=1, space="DRAM") as DR,
            tc.tile_pool(name="const", bufs=1) as CONST,
            tc.tile_pool(name="wp", bufs=1) as WP,
            tc.tile_pool(name="qk", bufs=1) as QK,
            tc.tile_pool(name="vres", bufs=1) as VR,
            tc.tile_pool(name="outT", bufs=1) as OT,
        ):
            # ---- DRAM bounces, AllGathers (overlap with qkv phase) ----
            wb = DR.tile([128, 1024], bf16, tag="wb", name="wb")
            agw = DR.tile([1024, 1024], bf16, tag="agw", name="agw")
            cb = DR.tile([80, 1024], bf16, tag="cb", name="cb")
            agc = DR.tile([640, 1024], bf16, tag="agc", name="agc")
            bb = DR.tile([H * 128, 1024], bf16, tag="bb", name="bb")
            btd = DR.tile([NCORES * H * 128, 1024], bf16, tag="btd", name="btd")

            nc.sync.dma_start(out=wb[:], in_=pack[ROW_W:ROW_W + 128, :])
            nc.sync.dma_start(out=cb[:], in_=pack[ROW_CE:ROW_CE + 80, :])
            nc.gpsimd.collective_compute(
                "AllGather", BYP, RG, [wb[:].opt()], [agw[:].opt()])
            nc.gpsimd.collective_compute(
                "AllGather", BYP, RG, [cb[:].opt()], [agc[:].opt()])

            # ---- constants from the pack ----
            eye_t = CONST.tile([128, 128], bf16, tag="eye", name="eye")
            nc.sync.dma_start(
                out=eye_t[:],
                in_=pack[ROW_EYE:ROW_EYE + 16, :].rearrange(
                    "a (b c) -> (a b) c", b=8))
            ones_t = CONST.tile([1, 128], bf16, tag="ones", name="ones")
            nc.sync.dma_start(out=ones_t[:], in_=pack[ROW_MISC:ROW_MISC + 1, 0:128])
            bpr_t = CONST.tile([1, DIM], bf16, tag="bpr", name="bpr")
            nc.sync.dma_start(out=bpr_t[:], in_=pack[ROW_MISC:ROW_MISC + 1, 128:640])
            ce_t = CONST.tile([128, H * KH * 128], bf16, tag="ce", name="ce")
            nc.sync.dma_start(
                out=ce_t[:], in_=agc[:].rearrange("(p r) c -> p (r c)", r=5))

            # weights from the gathered pack [512, 2048] = [wqkvT | wprojT]
            wq_t = [WP.tile([128, 3 * DIM], bf16, tag=f"wq{c}", name=f"wq{c}")
                    for c in range(4)]
            wp_t = [WP.tile([128, DIM], bf16, tag=f"wp{c}", name=f"wp{c}")
                    for c in range(4)]
            for c in range(4):
                src = agw[c * 256:(c + 1) * 256, :].rearrange(
                    "(p r) c -> p (r c)", r=2)
                nc.sync.dma_start(out=wq_t[c][:], in_=src[:, 0:3 * DIM])
                nc.sync.dma_start(out=wp_t[c][:], in_=src[:, 3 * DIM:4 * DIM])

            qk_t = [QK.tile([128, TOK], bf16, tag=f"qk{o}", name=f"qk{o}")
                    for o in range(8)]
            v_t = [VR.tile([128, VW], bf16, tag=f"v{t}", name=f"v{t}")
                   for t in range(NT)]
            oT_t = [OT.tile([128, N], bf16, tag=f"oT{b}_{c}", name=f"oT{b}_{c}")
                    for b in range(BPC) for c in range(4)]
            bpb_t = CONST.tile([128, DIM], f32, tag="bpb", name="bpb")

            # ---------------- phase 1: qkv projections ----------------
            with (
                tc.tile_pool(name="xw", bufs=1) as XW,
                tc.tile_pool(name="ps1", bufs=4, space="PSUM") as PS1,
            ):
                # bproj broadcast to 128 partitions via outer product
                psb0 = PS1.tile([128, DIM], f32, tag="ps1", name="ps1")
                nc.tensor.matmul(psb0[:], ones_t[:], bpr_t[:],
                                 start=True, stop=True)
                nc.vector.tensor_copy(bpb_t[:], psb0[:])

                xT_t = [XW.tile([128, TOK], bf16, tag=f"x{c}", name=f"x{c}")
                        for c in range(4)]
                for c in range(4):
                    nc.sync.dma_start(
                        out=xT_t[c][:],
                        in_=pack[c * 256:(c + 1) * 256, :].rearrange(
                            "(p r) c -> p (r c)", r=2))

                # q,k transposed: qkvT[o_tile, tok] ; o tiles 0..7 cover q,k
                for o in range(8):
                    for t in range(4):           # tok chunks of 512
                        ps = PS1.tile([128, 512], f32, tag="ps1", name="ps1")
                        for c in range(4):
                            nc.tensor.matmul(
                                ps[:], wq_t[c][:, o * 128:(o + 1) * 128],
                                xT_t[c][:, t * 512:(t + 1) * 512],
                                start=(c == 0), stop=(c == 3))
                        nc.vector.tensor_copy(qk_t[o][:, t * 512:(t + 1) * 512], ps[:])
                # v natural: [tok_tile, vch] -> packed per head with ones col
                for t in range(NT):
                    ps = PS1.tile([128, 512], f32, tag="ps1", name="ps1")
                    for c in range(4):
                        nc.tensor.matmul(
                            ps[:], xT_t[c][:, t * 128:(t + 1) * 128],
                            wq_t[c][:, 2 * DIM:3 * DIM],
                            start=(c == 0), stop=(c == 3))
                    dst = v_t[t][:, 0:VW].rearrange("p (h s) -> p h s", s=HD + 1)
                    nc.vector.tensor_copy(
                        dst[:, :, 0:HD],
                        ps[:].rearrange("p (h s) -> p h s", s=HD))
                    nc.vector.memset(dst[:, :, HD:HD + 1], 1.0)

            # ------- phase 1.5: m-row-sharded bias mixture + AllGather -----
            # This core holds HstackT_k[128c:128c+128, :] for all k in its
            # pack; it computes biasT[h][those rows] = sum_k c_hk * H_k for
            # ALL heads (scaled-eye stationaries ce_t[:, (h*5+k)*128:+128]),
            # then one AllGather assembles the full biasT across cores:
            # btd[mi*1024 + h*128 + p, :] = biasT[h][mi*128 + p, :].
            with (
                tc.tile_pool(name="hbt", bufs=5) as HBT,
                tc.tile_pool(name="bw", bufs=3) as BW,
                tc.tile_pool(name="psb", bufs=2, space="PSUM") as PSB,
            ):
                h_tiles = []
                for k in range(KH):
                    ht = HBT.tile([128, N], bf16, tag="ht", name="ht")
                    nc.sync.dma_start(
                        out=ht[:],
                        in_=pack[ROW_H + k * 128:ROW_H + (k + 1) * 128, :])
                    h_tiles.append(ht)
                for h in range(H):
                    psb = PSB.tile([128, N], f32, tag="psb", name="psb")
                    for chunk in range(2):
                        sl = slice(chunk * 512, (chunk + 1) * 512)
                        for k in range(KH):
                            idx = h * KH + k
                            nc.tensor.matmul(
                                psb[:, sl],
                                ce_t[:, idx * 128:(idx + 1) * 128],
                                h_tiles[k][:, sl],
                                start=(k == 0), stop=(k == KH - 1))
                    bt = BW.tile([128, N], bf16, tag="bt", name="bt")
                    nc.vector.tensor_copy(bt[:], psb[:])
                    nc.sync.dma_start(
                        out=bb[h * 128:(h + 1) * 128, :], in_=bt[:])
                nc.gpsimd.collective_compute(
                    "AllGather", BYP, RG, [bb[:].opt()], [btd[:].opt()])

            # ---------------- phase 2: attention ----------------
            with (
                tc.tile_pool(name="biasp", bufs=18) as BP,
                tc.tile_pool(name="pp", bufs=14) as PP,
                tc.tile_pool(name="nrm", bufs=4) as NRM,
                tc.tile_pool(name="ysb", bufs=3) as YSB,
                tc.tile_pool(name="pss", bufs=2, space="PSUM") as PSS,
                tc.tile_pool(name="pso", bufs=1, space="PSUM") as PSO,
                tc.tile_pool(name="psm", bufs=2, space="PSUM") as PSM,
            ):
                for h in range(H):
                    qt, po = qk_t[h // 2], (h % 2) * 64
                    kt = qk_t[4 + h // 2]
                    b_tiles = []
                    for mi in range(8):
                        bt = BP.tile([128, N], bf16, tag="bias", name="bias")
                        nc.sync.dma_start(
                            out=bt[:],
                            in_=btd[mi * 1024 + h * 128:
                                    mi * 1024 + (h + 1) * 128, :])
                        b_tiles.append(bt)
                    for b in range(BPC):
                        t0 = b * N
                        p_tiles = []
                        for mi in range(8):
                            ps = PSS.tile([128, N], f32, tag="pss", name="pss")
                            for nchunk in range(2):
                                sl = slice(nchunk * 512, (nchunk + 1) * 512)
                                nc.tensor.matmul(
                                    ps[:, sl],
                                    kt[po:po + 64, t0 + mi * 128: t0 + (mi + 1) * 128],
                                    qt[po:po + 64, t0 + nchunk * 512: t0 + (nchunk + 1) * 512],
                                    start=True, stop=False)
                                nc.tensor.matmul(
                                    ps[:, sl], eye_t[:], b_tiles[mi][:, sl],
                                    start=False, stop=True)
                            pt = PP.tile([128, N], bf16, tag="p", name="p")
                            nc.scalar.activation(pt[:], ps[:], EXP)
                            p_tiles.append(pt)
                        pso = PSO.tile([HD + 1, N], f32, tag="pso", name="pso")
                        for mi in range(8):
                            for nchunk in range(2):
                                sl = slice(nchunk * 512, (nchunk + 1) * 512)
                                nc.tensor.matmul(
                                    pso[:, sl],
                                    v_t[b * 8 + mi][:, h * (HD + 1):(h + 1) * (HD + 1)],
                                    p_tiles[mi][:, sl],
                                    start=(mi == 0), stop=(mi == 7))
                        # denominator -> broadcast -> reciprocal -> normalize
                        d_t = NRM.tile([1, N], bf16, tag="d", name="d")
                        nc.vector.tensor_copy(d_t[:], pso[64:65, :])
                        R_t = NRM.tile([64, N], f32, tag="R", name="R")
                        for nchunk in range(2):
                            sl = slice(nchunk * 512, (nchunk + 1) * 512)
                            psr = PSM.tile([64, 512], f32, tag="psm", name="psm")
                            nc.tensor.matmul(psr[:], ones_t[:, 0:64], d_t[:, sl],
                                             start=True, stop=True)
                            nc.vector.reciprocal(R_t[:, sl], psr[:])
                        nc.vector.tensor_tensor(
                            oT_t[b * 4 + h // 2][po:po + 64, :],
                            pso[0:64, :], R_t[:], MUL)
                # ---------------- phase 3: output projection ----------------
                for b in range(BPC):
                    for t in range(8):
                        psy = PSM.tile([128, 512], f32, tag="psm", name="psm")
                        for c in range(4):
                            nc.tensor.matmul(
                                psy[:],
                                oT_t[b * 4 + c][:, t * 128:(t + 1) * 128],
                                wp_t[c][:], start=(c == 0), stop=(c == 3))
                        yt = YSB.tile([128, DIM], f16, tag="y", name="y")
                        nc.vector.tensor_tensor(yt[:], psy[:], bpb_t[:], ADD)
                        nc.sync.dma_start(
                            out=y[b * N + t * 128: b * N + (t + 1) * 128, :],
                            in_=yt[:])
    nc.compile()
    return nc


def _prep_host(x, Hstack, hop_logits_attn, rel_alpha, Wqkv, Wproj, bproj):
    bf = ml_dtypes.bfloat16
    lg = hop_logits_attn - hop_logits_attn.max(-1, keepdims=True)
    w = np.exp(lg)
    w /= w.sum(-1, keepdims=True)                      # [H, KH]
    c = (rel_alpha[:, None] * w).astype(np.float32)    # [H, KH]
    # scaled-identity blocks [128, H*KH*128]
    ce = np.zeros((128, H * KH * 128), np.float32)
    eye128 = np.eye(128, dtype=np.float32)
    for h in range(H):
        for k in range(KH):
            idx = h * KH + k
            ce[:, idx * 128:(idx + 1) * 128] = c[h, k] * eye128
    ce = ce.astype(bf)

    HstackT = np.ascontiguousarray(
        Hstack.astype(np.float32).transpose(0, 2, 1)).astype(bf)  # [KH, N, N]
    wqkvT = np.ascontiguousarray(Wqkv.T).astype(np.float32).copy()
    wqkvT[:, :DIM] *= SCALE                            # fold q scaling
    wpack = np.concatenate(
        [wqkvT, np.ascontiguousarray(Wproj.T)], axis=1).astype(bf)  # [512, 2048]
    eye_bf = eye128.astype(bf)

    in_maps = []
    for i in range(NCORES):
        pk = np.zeros((PACK_ROWS, 1024), bf)
        xi = x[i * BPC:(i + 1) * BPC].reshape(TOK, DIM)
        pk[ROW_X:ROW_X + 1024] = np.ascontiguousarray(xi.T).astype(bf).reshape(1024, 1024)
        pk[ROW_H:ROW_H + 640] = HstackT[:, i * 128:(i + 1) * 128, :].reshape(640, 1024)
        pk[ROW_W:ROW_W + 128] = wpack[i * 64:(i + 1) * 64].reshape(128, 1024)
        pk[ROW_CE:ROW_CE + 80] = ce[i * 16:(i + 1) * 16].reshape(80, 1024)
        pk[ROW_EYE:ROW_EYE + 16] = eye_bf.reshape(16, 1024)
        pk[ROW_MISC, 0:128] = 1.0
        pk[ROW_MISC, 128:640] = bproj.astype(bf)
        in_maps.append(dict(pack=pk))
    return in_maps


def kernel(**inputs):
    from concourse.bass_utils import run_bass_kernel_spmd

    if "nc" not in _CACHE:
        _CACHE["nc"] = _build()
    nc = _CACHE["nc"]
    in_maps = _prep_host(
        np.asarray(inputs["x"], np.float32),
        np.asarray(inputs["Hstack"], np.float32),
        np.asarray(inputs["hop_logits_attn"], np.float32),
        np.asarray(inputs["rel_alpha"], np.float32),
        np.asarray(inputs["Wqkv"], np.float32),
        np.asarray(inputs["Wproj"], np.float32),
        np.asarray(inputs["bproj"], np.float32))
    res = run_bass_kernel_spmd(nc, in_maps, list(range(NCORES))).results
    out = np.concatenate(
        [r["y"].astype(np.float32).reshape(BPC, N, DIM) for r in res], axis=0)
    return out


# revision 16
# speedup vs baseline: 4.2179x; 1.1487x over previous
"""Trainium2 Bass kernel for nn_Attention_xxc (dense transformer attention
with hop-distance bias). Data-parallel over batch: 8 cores x 2 batches.

Transfer/dispatch-optimized v4. The metric is warm wall time of
run_bass_kernel_spmd; measured cost structure on this axon-tunneled path:
  - host->device upload ~13.4 ns/B, download ~13.4 ns/B (outputs also pay
    a zeros-donation upload), per-call pjit recompile ~0.4s unless the jax
    persistent compilation cache is on, device DMA ~0.23 ms/instruction +
    ~2.6 ns/B, collective (RDH) bytes ~free, device compute ~free.
Hence:
  - ONE packed bf16 input [1249, 1024] (~2.6MB) + u8 Hstack shard
    [640, 1024] (~0.65MB) per core instead of 20.3MB replicated tensors:
    xT + m-row shard of HstackT (u8-quantized, x255) + 1/8 of
    [Wqkv.T|Wproj.T] + 1/8 of the scaled-identity mixture blocks (with
    the 1/255 dequant folded in) + eye + ones/bproj row.
  - Device AllGathers rebuild weights and mixture blocks; each core
    computes biasT rows [128c, 128c+128) for ALL heads (its HstackT
    shard), and one AllGather assembles the full 16MB biasT.
  - DMAs are batched: one instruction per logical load/store group
    (weights, x, H, bias-write, per-head bias load, per-batch y store).
  - Output y in fp16 (download + zeros upload are the costliest bytes).
  - jax persistent compilation cache kills the per-call recompile.

Per-core compute layout (as the proven v1): q,k transposed bf16; v natural
with a ones column per head (denominator); S.T = k.q + bias.T via
identity-matmul PSUM accumulation; exp on ACT; AV -> out.T with row 64 =
denom; normalize via reciprocal broadcast; proj + bproj -> y.
"""
import sys

sys.path.insert(0, "/opt/trn_rl_repo")

import numpy as np
import ml_dtypes
import jax

# Persistent compilation cache: run_bass_kernel_spmd re-lowers and
# re-compiles its jit wrapper on every call (fresh closure -> jit cache
# miss); with the disk cache the per-call XLA/neuronx compile becomes a
# ~137KB cache hit instead of ~0.4s of recompilation.
jax.config.update("jax_compilation_cache_dir", "/tmp/jaxcache")
jax.config.update("jax_persistent_cache_min_compile_time_secs", 0)
jax.config.update("jax_persistent_cache_min_entry_size_bytes", 0)

B, N, DIM = 16, 1024, 512
H, HD, KH = 8, 64, 5
SCALE = HD ** -0.5
NCORES = 8
BPC = B // NCORES          # batches per core
TOK = BPC * N              # tokens per core = 2048

# packed bf16 input row offsets (1024-wide rows)
ROW_X = 0        # 1024 rows: xT [512, 2048]
ROW_W = 1024     # 128 rows: weight shard [64, 2048]
ROW_CE = 1152    # 80 rows: c_eye shard [16, 5120]
ROW_EYE = 1232   # 16 rows: eye [128, 128]
ROW_MISC = 1248  # 1 row: ones [0:128], bproj [128:640]
PACK_ROWS = 1249

_CACHE = {}


def _build():
    import concourse.bass as bass
    import concourse.bacc as bacc
    import concourse.mybir as mybir
    from concourse.tile import TileContext

    f32 = mybir.dt.float32
    bf16 = mybir.dt.bfloat16
    f16 = mybir.dt.float16
    u8 = mybir.dt.uint8
    EXP = mybir.ActivationFunctionType.Exp
    MUL = mybir.AluOpType.mult
    ADD = mybir.AluOpType.add
    BYP = mybir.AluOpType.bypass

    nc = bacc.Bacc(num_devices=NCORES)
    pack = nc.declare_dram_parameter("pack", [PACK_ROWS, 1024], bf16, isOutput=False)
    hu8 = nc.declare_dram_parameter("hu8", [KH * 128, 1024], u8, isOutput=False)
    y = nc.declare_dram_parameter("y", [TOK, DIM], f16, isOutput=True)

    NT = TOK // 128            # 16 token tiles
    VW = H * (HD + 1)          # 520: v row width with ones col per head
    RG = [list(range(NCORES))]

    with TileContext(nc) as tc:
        with (
            tc.tile_pool(name="dram", bufs=1, space="DRAM") as DR,
            tc.tile_pool(name="const", bufs=1) as CONST,
            tc.tile_pool(name="qk", bufs=1) as QK,
            tc.tile_pool(name="vres", bufs=1) as VR,
            tc.tile_pool(name="outT", bufs=1) as OT,
        ):
            # ---- DRAM bounces, AllGathers (overlap with qkv phase) ----
            wb = DR.tile([128, 1024], bf16, tag="wb", name="wb")
            agw = DR.tile([1024, 1024], bf16, tag="agw", name="agw")
            cb = DR.tile([80, 1024], bf16, tag="cb", name="cb")
            agc = DR.tile([640, 1024], bf16, tag="agc", name="agc")
            bb = DR.tile([H * 128, 1024], bf16, tag="bb", name="bb")
            btd = DR.tile([NCORES, H, 128, 1024], bf16, tag="btd", name="btd")

            nc.sync.dma_start(out=wb[:], in_=pack[ROW_W:ROW_W + 128, :])
            nc.sync.dma_start(out=cb[:], in_=pack[ROW_CE:ROW_CE + 80, :])
            nc.gpsimd.collective_compute(
                "AllGather", BYP, RG, [wb[:].opt()], [agw[:].opt()])
            nc.gpsimd.collective_compute(
                "AllGather", BYP, RG, [cb[:].opt()], [agc[:].opt()])

            # ---- constants (batched loads) ----
            eye_t = CONST.tile([128, 128], bf16, tag="eye", name="eye")
            nc.sync.dma_start(
                out=eye_t[:],
                in_=pack[ROW_EYE:ROW_EYE + 16, :].rearrange(
                    "a (b c) -> (a b) c", b=8))
            misc_t = CONST.tile([1, 640], bf16, tag="misc", name="misc")
            nc.sync.dma_start(out=misc_t[:], in_=pack[ROW_MISC:ROW_MISC + 1, 0:640])
            ones_t = misc_t[:, 0:128]
            bpr_t = misc_t[:, 128:640]
            ce_t = CONST.tile([128, H * KH * 128], bf16, tag="ce", name="ce")
            nc.sync.dma_start(
                out=ce_t[:], in_=agc[:].rearrange("(p r) c -> p (r c)", r=5))
            # all weights in one DMA: wall[:, c*2048:(c+1)*2048] holds
            # wqkvT rows [128c,128c+128) (cols 0:1536) | wprojT (cols 1536:)
            wall = CONST.tile([128, 8192], bf16, tag="wall", name="wall")
            nc.sync.dma_start(
                out=wall[:].rearrange("p (c r n) -> p c r n", c=4, r=2),
                in_=agw[:].rearrange("(c p r) n -> p c r n", c=4, r=2))
            wq = [wall[:, c * 2048:c * 2048 + 1536] for c in range(4)]
            wp = [wall[:, c * 2048 + 1536:(c + 1) * 2048] for c in range(4)]

            qk_t = [QK.tile([128, TOK], bf16, tag=f"qk{o}", name=f"qk{o}")
                    for o in range(8)]
            v_t = [VR.tile([128, VW], bf16, tag=f"v{t}", name=f"v{t}")
                   for t in range(NT)]
            oT_t = [OT.tile([128, N], bf16, tag=f"oT{b}_{c}", name=f"oT{b}_{c}")
                    for b in range(BPC) for c in range(4)]
            bpb_t = CONST.tile([128, DIM], f32, tag="bpb", name="bpb")

            # ---------------- phase 1: qkv projections ----------------
            with (
                tc.tile_pool(name="xw", bufs=1) as XW,
                tc.tile_pool(name="ps1", bufs=4, space="PSUM") as PS1,
            ):
                # bproj broadcast to 128 partitions via outer product
                psb0 = PS1.tile([128, DIM], f32, tag="ps1", name="ps1")
                nc.tensor.matmul(psb0[:], ones_t, bpr_t,
                                 start=True, stop=True)
                nc.vector.tensor_copy(bpb_t[:], psb0[:])

                # all of xT in one DMA: xall[:, c*2048 + col] = xT[128c+p, col]
                xall = XW.tile([128, 8192], bf16, tag="xall", name="xall")
                nc.sync.dma_start(
                    out=xall[:].rearrange("p (c r n) -> p c r n", c=4, r=2),
                    in_=pack[ROW_X:ROW_X + 1024, :].rearrange(
                        "(c p r) n -> p c r n", c=4, r=2))
                xT = [xall[:, c * 2048:(c + 1) * 2048] for c in range(4)]

                # q,k transposed: qkvT[o_tile, tok] ; o tiles 0..7 cover q,k
                for o in range(8):
                    for t in range(4):           # tok chunks of 512
                        ps = PS1.tile([128, 512], f32, tag="ps1", name="ps1")
                        for c in range(4):
                            nc.tensor.matmul(
                                ps[:], wq[c][:, o * 128:(o + 1) * 128],
                                xT[c][:, t * 512:(t + 1) * 512],
                                start=(c == 0), stop=(c == 3))
                        nc.vector.tensor_copy(qk_t[o][:, t * 512:(t + 1) * 512], ps[:])
                # v natural: [tok_tile, vch] -> packed per head with ones col
                for t in range(NT):
                    ps = PS1.tile([128, 512], f32, tag="ps1", name="ps1")
                    for c in range(4):
                        nc.tensor.matmul(
                            ps[:], xT[c][:, t * 128:(t + 1) * 128],
                            wq[c][:, 2 * DIM:3 * DIM],
                            start=(c == 0), stop=(c == 3))
                    dst = v_t[t][:, 0:VW].rearrange("p (h s) -> p h s", s=HD + 1)
                    nc.vector.tensor_copy(
                        dst[:, :, 0:HD],
                        ps[:].rearrange("p (h s) -> p h s", s=HD))
                    nc.vector.memset(dst[:, :, HD:HD + 1], 1.0)

            # ------- phase 1.5: m-row-sharded bias mixture + AllGather -----
            # This core holds u8 HstackT_k[128c:128c+128, :]*255 for all k;
            # it computes biasT[h][those rows] for ALL heads via scaled-eye
            # stationaries ce_t (1/255 dequant folded in), then one
            # AllGather assembles the full biasT:
            # btd[c, h, p, :] = biasT[h][128c + p, :].
            with (
                tc.tile_pool(name="hbt", bufs=1) as HBT,
                tc.tile_pool(name="bw", bufs=1) as BW,
                tc.tile_pool(name="psb", bufs=2, space="PSUM") as PSB,
            ):
                hu = HBT.tile([128, KH * 1024], u8, tag="hu", name="hu")
                nc.sync.dma_start(
                    out=hu[:].rearrange("p (k n) -> p k n", k=KH),
                    in_=hu8[:].rearrange("(k p) n -> p k n", p=128))
                hall = HBT.tile([128, KH * 1024], bf16, tag="hall", name="hall")
                nc.vector.tensor_copy(hall[:], hu[:])
                ball = BW.tile([128, H * 1024], bf16, tag="ball", name="ball")
                for h in range(H):
                    psb = PSB.tile([128, N], f32, tag="psb", name="psb")
                    for chunk in range(2):
                        sl = slice(chunk * 512, (chunk + 1) * 512)
                        for k in range(KH):
                            idx = h * KH + k
                            nc.tensor.matmul(
                                psb[:, sl],
                                ce_t[:, idx * 128:(idx + 1) * 128],
                                hall[:, k * 1024:k * 1024 + 1024][:, sl],
                                start=(k == 0), stop=(k == KH - 1))
                    nc.vector.tensor_copy(
                        ball[:, h * 1024:(h + 1) * 1024], psb[:])
                nc.sync.dma_start(
                    out=bb[:].rearrange("(h p) n -> p h n", p=128),
                    in_=ball[:].rearrange("p (h n) -> p h n", h=H))
                nc.gpsimd.collective_compute(
                    "AllGather", BYP, RG, [bb[:].opt()], [btd[:].opt()])

            # ---------------- phase 2: attention ----------------
            with (
                tc.tile_pool(name="biasp", bufs=2) as BP,
                tc.tile_pool(name="pp", bufs=14) as PP,
                tc.tile_pool(name="nrm", bufs=4) as NRM,
                tc.tile_pool(name="ysb", bufs=2) as YSB,
                tc.tile_pool(name="pss", bufs=2, space="PSUM") as PSS,
                tc.tile_pool(name="pso", bufs=1, space="PSUM") as PSO,
                tc.tile_pool(name="psm", bufs=2, space="PSUM") as PSM,
            ):
                for h in range(H):
                    qt, po = qk_t[h // 2], (h % 2) * 64
                    kt = qk_t[4 + h // 2]
                    # all 8 bias m-tiles for this head in one DMA
                    b_all = BP.tile([128, 8 * 1024], bf16, tag="bias", name="bias")
                    nc.sync.dma_start(
                        out=b_all[:].rearrange("p (m c) -> p m c", m=8),
                        in_=btd[:, h, :, :].rearrange("m p c -> p m c"))
                    for b in range(BPC):
                        t0 = b * N
                        p_tiles = []
                        for mi in range(8):
                            ps = PSS.tile([128, N], f32, tag="pss", name="pss")
                            for nchunk in range(2):
                                sl = slice(nchunk * 512, (nchunk + 1) * 512)
                                nc.tensor.matmul(
                                    ps[:, sl],
                                    kt[po:po + 64, t0 + mi * 128: t0 + (mi + 1) * 128],
                                    qt[po:po + 64, t0 + nchunk * 512: t0 + (nchunk + 1) * 512],
                                    start=True, stop=False)
                                nc.tensor.matmul(
                                    ps[:, sl], eye_t[:],
                                    b_all[:, mi * 1024 + nchunk * 512:
                                          mi * 1024 + (nchunk + 1) * 512],
                                    start=False, stop=True)
                            pt = PP.tile([128, N], bf16, tag="p", name="p")
                            nc.scalar.activation(pt[:], ps[:], EXP)
                            p_tiles.append(pt)
                        pso = PSO.tile([HD + 1, N], f32, tag="pso", name="pso")
                        for mi in range(8):
                            for nchunk in range(2):
                                sl = slice(nchunk * 512, (nchunk + 1) * 512)
                                nc.tensor.matmul(
                                    pso[:, sl],
                                    v_t[b * 8 + mi][:, h * (HD + 1):(h + 1) * (HD + 1)],
                                    p_tiles[mi][:, sl],
                                    start=(mi == 0), stop=(mi == 7))
                        # denominator -> broadcast -> reciprocal -> normalize
                        d_t = NRM.tile([1, N], bf16, tag="d", name="d")
                        nc.vector.tensor_copy(d_t[:], pso[64:65, :])
                        R_t = NRM.tile([64, N], f32, tag="R", name="R")
                        for nchunk in range(2):
                            sl = slice(nchunk * 512, (nchunk + 1) * 512)
                            psr = PSM.tile([64, 512], f32, tag="psm", name="psm")
                            nc.tensor.matmul(psr[:], ones_t[:, 0:64], d_t[:, sl],
                                             start=True, stop=True)
                            nc.vector.reciprocal(R_t[:, sl], psr[:])
                        nc.vector.tensor_tensor(
                            oT_t[b * 4 + h // 2][po:po + 64, :],
                            pso[0:64, :], R_t[:], MUL)
                # ---------------- phase 3: output projection ----------------
                for b in range(BPC):
                    yt = YSB.tile([128, 8 * DIM], f16, tag="y", name="y")
                    for t in range(8):
                        psy = PSM.tile([128, 512], f32, tag="psm", name="psm")
                        for c in range(4):
                            nc.tensor.matmul(
                                psy[:],
                                oT_t[b * 4 + c][:, t * 128:(t + 1) * 128],
                                wp[c], start=(c == 0), stop=(c == 3))
                        nc.vector.tensor_tensor(
                            yt[:, t * DIM:(t + 1) * DIM], psy[:], bpb_t[:], ADD)
                    nc.sync.dma_start(
                        out=y[b * N:(b + 1) * N, :].rearrange(
                            "(t p) c -> p t c", p=128),
                        in_=yt[:].rearrange("p (t c) -> p t c", t=8))
    nc.compile()
    return nc


def _prep_host(x, Hstack, hop_logits_attn, rel_alpha, Wqkv, Wproj, bproj):
    bf = ml_dtypes.bfloat16
    lg = hop_logits_attn - hop_logits_attn.max(-1, keepdims=True)
    w = np.exp(lg)
    w /= w.sum(-1, keepdims=True)                      # [H, KH]
    # 1/255 dequant for the u8 Hstack folded into the mixture scales
    c = (rel_alpha[:, None] * w).astype(np.float32) / 255.0   # [H, KH]
    # scaled-identity blocks [128, H*KH*128]
    ce = np.zeros((128, H * KH * 128), np.float32)
    eye128 = np.eye(128, dtype=np.float32)
    for h in range(H):
        for k in range(KH):
            idx = h * KH + k
            ce[:, idx * 128:(idx + 1) * 128] = c[h, k] * eye128
    ce = ce.astype(bf)

    HstackT = np.ascontiguousarray(
        Hstack.astype(np.float32).transpose(0, 2, 1))  # [KH, N, N]
    Hu8 = np.clip(np.rint(HstackT * 255.0), 0, 255).astype(np.uint8)
    wqkvT = np.ascontiguousarray(Wqkv.T).astype(np.float32).copy()
    wqkvT[:, :DIM] *= SCALE                            # fold q scaling
    wpack = np.concatenate(
        [wqkvT, np.ascontiguousarray(Wproj.T)], axis=1).astype(bf)  # [512, 2048]
    eye_bf = eye128.astype(bf)

    in_maps = []
    for i in range(NCORES):
        pk = np.zeros((PACK_ROWS, 1024), bf)
        xi = x[i * BPC:(i + 1) * BPC].reshape(TOK, DIM)
        pk[ROW_X:ROW_X + 1024] = np.ascontiguousarray(xi.T).astype(bf).reshape(1024, 1024)
        pk[ROW_W:ROW_W + 128] = wpack[i * 64:(i + 1) * 64].reshape(128, 1024)
        pk[ROW_CE:ROW_CE + 80] = ce[i * 16:(i + 1) * 16].reshape(80, 1024)
        pk[ROW_EYE:ROW_EYE + 16] = eye_bf.reshape(16, 1024)
        pk[ROW_MISC, 0:128] = 1.0
        pk[ROW_MISC, 128:640] = bproj.astype(bf)
        hsh = Hu8[:, i * 128:(i + 1) * 128, :].reshape(KH * 128, 1024)
        in_maps.append(dict(pack=pk, hu8=np.ascontiguousarray(hsh)))
    return in_maps


def kernel(**inputs):
    from concourse.bass_utils import run_bass_kernel_spmd

    if "nc" not in _CACHE:
        _CACHE["nc"] = _build()
    nc = _CACHE["nc"]
    in_maps = _prep_host(
        np.asarray(inputs["x"], np.float32),
        np.asarray(inputs["Hstack"], np.float32),
        np.asarray(inputs["hop_logits_attn"], np.float32),
        np.asarray(inputs["rel_alpha"], np.float32),
        np.asarray(inputs["Wqkv"], np.float32),
        np.asarray(inputs["Wproj"], np.float32),
        np.asarray(inputs["bproj"], np.float32))
    res = run_bass_kernel_spmd(nc, in_maps, list(range(NCORES))).results
    out = np.concatenate(
        [r["y"].astype(np.float32).reshape(BPC, N, DIM) for r in res], axis=0)
    return out


# revision 23
# speedup vs baseline: 4.2623x; 1.0105x over previous
"""Trainium2 Bass kernel for nn_Attention_xxc (dense transformer attention
with hop-distance bias). Data-parallel over batch: 8 cores x 2 batches.

Transfer/dispatch-optimized v4. The metric is warm wall time of
run_bass_kernel_spmd; measured cost structure on this axon-tunneled path:
  - host->device upload ~13.4 ns/B, download ~13.4 ns/B (outputs also pay
    a zeros-donation upload), per-call pjit recompile ~0.4s unless the jax
    persistent compilation cache is on, device DMA ~0.23 ms/instruction +
    ~2.6 ns/B, collective (RDH) bytes ~free, device compute ~free.
Hence:
  - ONE packed bf16 input [1249, 1024] (~2.6MB) + u8 Hstack shard
    [640, 1024] (~0.65MB) per core instead of 20.3MB replicated tensors:
    xT + m-row shard of HstackT (u8-quantized, x255) + 1/8 of
    [Wqkv.T|Wproj.T] + 1/8 of the scaled-identity mixture blocks (with
    the 1/255 dequant folded in) + eye + ones/bproj row.
  - Device AllGathers rebuild weights and mixture blocks; each core
    computes biasT rows [128c, 128c+128) for ALL heads (its HstackT
    shard), and one AllGather assembles the full 16MB biasT.
  - DMAs are batched: one instruction per logical load/store group
    (weights, x, H, bias-write, per-head bias load, per-batch y store).
  - Output y in fp16 (download + zeros upload are the costliest bytes).
  - jax persistent compilation cache kills the per-call recompile.

Per-core compute layout (as the proven v1): q,k transposed bf16; v natural
with a ones column per head (denominator); S.T = k.q + bias.T via
identity-matmul PSUM accumulation; exp on ACT; AV -> out.T with row 64 =
denom; normalize via reciprocal broadcast; proj + bproj -> y.
"""
import sys

sys.path.insert(0, "/opt/trn_rl_repo")

import numpy as np
import ml_dtypes
import jax

# Persistent compilation cache: run_bass_kernel_spmd re-lowers and
# re-compiles its jit wrapper on every call (fresh closure -> jit cache
# miss); with the disk cache the per-call XLA/neuronx compile becomes a
# ~137KB cache hit instead of ~0.4s of recompilation.
jax.config.update("jax_compilation_cache_dir", "/tmp/jaxcache")
jax.config.update("jax_persistent_cache_min_compile_time_secs", 0)
jax.config.update("jax_persistent_cache_min_entry_size_bytes", 0)

B, N, DIM = 16, 1024, 512
H, HD, KH = 8, 64, 5
SCALE = HD ** -0.5
NCORES = 8
BPC = B // NCORES          # batches per core
TOK = BPC * N              # tokens per core = 2048

# packed bf16 input row offsets (1024-wide rows)
ROW_X = 0        # 1024 rows: xT [512, 2048]
ROW_W = 1024     # 128 rows: weight shard [64, 2048]
ROW_CE = 1152    # 80 rows: c_eye shard [16, 5120]
ROW_MISC = 1232  # 1 row: ones [0:128], bproj [128:640]
PACK_ROWS = 1233

_CACHE = {}


def _build():
    import concourse.bass as bass
    import concourse.bacc as bacc
    import concourse.mybir as mybir
    from concourse.tile import TileContext

    f32 = mybir.dt.float32
    bf16 = mybir.dt.bfloat16
    f16 = mybir.dt.float16
    u8 = mybir.dt.uint8
    EXP = mybir.ActivationFunctionType.Exp
    MUL = mybir.AluOpType.mult
    ADD = mybir.AluOpType.add
    BYP = mybir.AluOpType.bypass

    nc = bacc.Bacc(num_devices=NCORES)
    pack = nc.declare_dram_parameter("pack", [PACK_ROWS, 1024], bf16, isOutput=False)
    hu8 = nc.declare_dram_parameter("hu8", [KH * 128, 1024], u8, isOutput=False)
    y = nc.declare_dram_parameter("y", [TOK, DIM], f16, isOutput=True)

    NT = TOK // 128            # 16 token tiles
    VW = H * (HD + 1)          # 520: v row width with ones col per head
    RG = [list(range(NCORES))]

    with TileContext(nc) as tc:
        with (
            tc.tile_pool(name="dram", bufs=1, space="DRAM") as DR,
            tc.tile_pool(name="const", bufs=1) as CONST,
            tc.tile_pool(name="qk", bufs=1) as QK,
            tc.tile_pool(name="vres", bufs=1) as VR,
            tc.tile_pool(name="outT", bufs=1) as OT,
        ):
            # ---- DRAM bounces, AllGathers (overlap with qkv phase) ----
            wb = DR.tile([128, 1024], bf16, tag="wb", name="wb")
            agw = DR.tile([1024, 1024], bf16, tag="agw", name="agw")
            cb = DR.tile([80, 1024], bf16, tag="cb", name="cb")
            agc = DR.tile([640, 1024], bf16, tag="agc", name="agc")
            bb = DR.tile([H * 128, 1024], bf16, tag="bb", name="bb")
            btd = DR.tile([NCORES, H, 128, 1024], bf16, tag="btd", name="btd")

            nc.sync.dma_start(out=wb[:], in_=pack[ROW_W:ROW_W + 128, :])
            nc.sync.dma_start(out=cb[:], in_=pack[ROW_CE:ROW_CE + 80, :])
            nc.gpsimd.collective_compute(
                "AllGather", BYP, RG, [wb[:].opt()], [agw[:].opt()])
            nc.gpsimd.collective_compute(
                "AllGather", BYP, RG, [cb[:].opt()], [agc[:].opt()])

            # ---- constants (batched loads) ----
            misc_t = CONST.tile([1, 640], bf16, tag="misc", name="misc")
            nc.sync.dma_start(out=misc_t[:], in_=pack[ROW_MISC:ROW_MISC + 1, 0:640])
            ones_t = misc_t[:, 0:128]
            bpr_t = misc_t[:, 128:640]
            ce_t = CONST.tile([128, H * KH * 128], bf16, tag="ce", name="ce")
            nc.sync.dma_start(
                out=ce_t[:], in_=agc[:].rearrange("(p r) c -> p (r c)", r=5))
            # all weights in one DMA: wall[:, c*2048:(c+1)*2048] holds
            # wqkvT rows [128c,128c+128) (cols 0:1536) | wprojT (cols 1536:)
            wall = CONST.tile([128, 8192], bf16, tag="wall", name="wall")
            nc.sync.dma_start(
                out=wall[:].rearrange("p (c r n) -> p c r n", c=4, r=2),
                in_=agw[:].rearrange("(c p r) n -> p c r n", c=4, r=2))
            wq = [wall[:, c * 2048:c * 2048 + 1536] for c in range(4)]
            wp = [wall[:, c * 2048 + 1536:(c + 1) * 2048] for c in range(4)]

            qk_t = [QK.tile([128, TOK], bf16, tag=f"qk{o}", name=f"qk{o}")
                    for o in range(8)]
            v_t = [VR.tile([128, VW], bf16, tag=f"v{t}", name=f"v{t}")
                   for t in range(NT)]
            oT_t = [OT.tile([128, N], bf16, tag=f"oT{b}_{c}", name=f"oT{b}_{c}")
                    for b in range(BPC) for c in range(4)]
            bpb_t = CONST.tile([128, DIM], f32, tag="bpb", name="bpb")

            # ---------------- phase 1: qkv projections ----------------
            with (
                tc.tile_pool(name="xw", bufs=1) as XW,
                tc.tile_pool(name="ps1", bufs=4, space="PSUM") as PS1,
            ):
                # bproj broadcast to 128 partitions via outer product
                psb0 = PS1.tile([128, DIM], f32, tag="ps1", name="ps1")
                nc.tensor.matmul(psb0[:], ones_t, bpr_t,
                                 start=True, stop=True)
                nc.vector.tensor_copy(bpb_t[:], psb0[:])

                # all of xT in one DMA: xall[:, c*2048 + col] = xT[128c+p, col]
                xall = XW.tile([128, 8192], bf16, tag="xall", name="xall")
                nc.sync.dma_start(
                    out=xall[:].rearrange("p (c r n) -> p c r n", c=4, r=2),
                    in_=pack[ROW_X:ROW_X + 1024, :].rearrange(
                        "(c p r) n -> p c r n", c=4, r=2))
                xT = [xall[:, c * 2048:(c + 1) * 2048] for c in range(4)]

                # q,k transposed: qkvT[o_tile, tok] ; o tiles 0..7 cover q,k
                for o in range(8):
                    for t in range(4):           # tok chunks of 512
                        ps = PS1.tile([128, 512], f32, tag="ps1", name="ps1")
                        for c in range(4):
                            nc.tensor.matmul(
                                ps[:], wq[c][:, o * 128:(o + 1) * 128],
                                xT[c][:, t * 512:(t + 1) * 512],
                                start=(c == 0), stop=(c == 3))
                        nc.vector.tensor_copy(qk_t[o][:, t * 512:(t + 1) * 512], ps[:])
                # v natural: [tok_tile, vch] -> packed per head with ones col
                for t in range(NT):
                    ps = PS1.tile([128, 512], f32, tag="ps1", name="ps1")
                    for c in range(4):
                        nc.tensor.matmul(
                            ps[:], xT[c][:, t * 128:(t + 1) * 128],
                            wq[c][:, 2 * DIM:3 * DIM],
                            start=(c == 0), stop=(c == 3))
                    dst = v_t[t][:, 0:VW].rearrange("p (h s) -> p h s", s=HD + 1)
                    nc.vector.tensor_copy(
                        dst[:, :, 0:HD],
                        ps[:].rearrange("p (h s) -> p h s", s=HD))
                    nc.vector.memset(dst[:, :, HD:HD + 1], 1.0)

            # ------- phase 1.5: m-row-sharded bias mixture + AllGather -----
            # This core holds u8 HstackT_k[128c:128c+128, :]*255 for all k;
            # it computes biasT[h][those rows] for ALL heads via scaled-eye
            # stationaries ce_t (1/255 dequant folded in), then one
            # AllGather assembles the full biasT:
            # btd[c, h, p, :] = biasT[h][128c + p, :].
            with (
                tc.tile_pool(name="hbt", bufs=1) as HBT,
                tc.tile_pool(name="bw", bufs=1) as BW,
                tc.tile_pool(name="psb", bufs=2, space="PSUM") as PSB,
            ):
                hu = HBT.tile([128, KH * 1024], u8, tag="hu", name="hu")
                nc.sync.dma_start(
                    out=hu[:].rearrange("p (k n) -> p k n", k=KH),
                    in_=hu8[:].rearrange("(k p) n -> p k n", p=128))
                hall = HBT.tile([128, KH * 1024], bf16, tag="hall", name="hall")
                nc.vector.tensor_copy(hall[:], hu[:])
                ball = BW.tile([128, H * 1024], bf16, tag="ball", name="ball")
                for h in range(H):
                    psb = PSB.tile([128, N], f32, tag="psb", name="psb")
                    for chunk in range(2):
                        sl = slice(chunk * 512, (chunk + 1) * 512)
                        for k in range(KH):
                            idx = h * KH + k
                            nc.tensor.matmul(
                                psb[:, sl],
                                ce_t[:, idx * 128:(idx + 1) * 128],
                                hall[:, k * 1024:k * 1024 + 1024][:, sl],
                                start=(k == 0), stop=(k == KH - 1))
                    # store exp(bias): scores use P = exp(S) * exp(bias)
                    nc.scalar.activation(
                        ball[:, h * 1024:(h + 1) * 1024], psb[:], EXP)
                nc.sync.dma_start(
                    out=bb[:].rearrange("(h p) n -> p h n", p=128),
                    in_=ball[:].rearrange("p (h n) -> p h n", h=H))
                nc.gpsimd.collective_compute(
                    "AllGather", BYP, RG, [bb[:].opt()], [btd[:].opt()])

            # ---------------- phase 2: attention ----------------
            with (
                tc.tile_pool(name="biasp", bufs=2) as BP,
                tc.tile_pool(name="pp", bufs=14) as PP,
                tc.tile_pool(name="nrm", bufs=4) as NRM,
                tc.tile_pool(name="ysb", bufs=2) as YSB,
                tc.tile_pool(name="pss", bufs=2, space="PSUM") as PSS,
                tc.tile_pool(name="pso", bufs=1, space="PSUM") as PSO,
                tc.tile_pool(name="psm", bufs=2, space="PSUM") as PSM,
            ):
                for h in range(H):
                    qt, po = qk_t[h // 2], (h % 2) * 64
                    kt = qk_t[4 + h // 2]
                    # all 8 exp-bias m-tiles for this head in one DMA
                    b_all = BP.tile([128, 8 * 1024], bf16, tag="bias", name="bias")
                    nc.sync.dma_start(
                        out=b_all[:].rearrange("p (m c) -> p m c", m=8),
                        in_=btd[:, h, :, :].rearrange("m p c -> p m c"))
                    for b in range(BPC):
                        t0 = b * N
                        p_tiles = []
                        for mi in range(8):
                            ps = PSS.tile([128, N], f32, tag="pss", name="pss")
                            for nchunk in range(2):
                                sl = slice(nchunk * 512, (nchunk + 1) * 512)
                                nc.tensor.matmul(
                                    ps[:, sl],
                                    kt[po:po + 64, t0 + mi * 128: t0 + (mi + 1) * 128],
                                    qt[po:po + 64, t0 + nchunk * 512: t0 + (nchunk + 1) * 512],
                                    start=True, stop=True)
                            pt = PP.tile([128, N], bf16, tag="p", name="p")
                            nc.scalar.activation(pt[:], ps[:], EXP)
                            nc.vector.tensor_tensor(
                                pt[:], pt[:],
                                b_all[:, mi * 1024:(mi + 1) * 1024], MUL)
                            p_tiles.append(pt)
                        pso = PSO.tile([HD + 1, N], f32, tag="pso", name="pso")
                        for mi in range(8):
                            for nchunk in range(2):
                                sl = slice(nchunk * 512, (nchunk + 1) * 512)
                                nc.tensor.matmul(
                                    pso[:, sl],
                                    v_t[b * 8 + mi][:, h * (HD + 1):(h + 1) * (HD + 1)],
                                    p_tiles[mi][:, sl],
                                    start=(mi == 0), stop=(mi == 7))
                        # denominator -> broadcast -> reciprocal -> normalize
                        d_t = NRM.tile([1, N], bf16, tag="d", name="d")
                        nc.vector.tensor_copy(d_t[:], pso[64:65, :])
                        R_t = NRM.tile([64, N], f32, tag="R", name="R")
                        for nchunk in range(2):
                            sl = slice(nchunk * 512, (nchunk + 1) * 512)
                            psr = PSM.tile([64, 512], f32, tag="psm", name="psm")
                            nc.tensor.matmul(psr[:], ones_t[:, 0:64], d_t[:, sl],
                                             start=True, stop=True)
                            nc.vector.reciprocal(R_t[:, sl], psr[:])
                        nc.vector.tensor_tensor(
                            oT_t[b * 4 + h // 2][po:po + 64, :],
                            pso[0:64, :], R_t[:], MUL)
                # ---------------- phase 3: output projection ----------------
                for b in range(BPC):
                    yt = YSB.tile([128, 8 * DIM], f16, tag="y", name="y")
                    for t in range(8):
                        psy = PSM.tile([128, 512], f32, tag="psm", name="psm")
                        for c in range(4):
                            nc.tensor.matmul(
                                psy[:],
                                oT_t[b * 4 + c][:, t * 128:(t + 1) * 128],
                                wp[c], start=(c == 0), stop=(c == 3))
                        nc.vector.tensor_tensor(
                            yt[:, t * DIM:(t + 1) * DIM], psy[:], bpb_t[:], ADD)
                    nc.sync.dma_start(
                        out=y[b * N:(b + 1) * N, :].rearrange(
                            "(t p) c -> p t c", p=128),
                        in_=yt[:].rearrange("p (t c) -> p t c", t=8))
    nc.compile()
    return nc


def _prep_host(x, Hstack, hop_logits_attn, rel_alpha, Wqkv, Wproj, bproj):
    bf = ml_dtypes.bfloat16
    lg = hop_logits_attn - hop_logits_attn.max(-1, keepdims=True)
    w = np.exp(lg)
    w /= w.sum(-1, keepdims=True)                      # [H, KH]
    # 1/255 dequant for the u8 Hstack folded into the mixture scales
    c = (rel_alpha[:, None] * w).astype(np.float32) / 255.0   # [H, KH]
    # scaled-identity blocks [128, H*KH*128]
    ce = np.zeros((128, H * KH * 128), np.float32)
    eye128 = np.eye(128, dtype=np.float32)
    for h in range(H):
        for k in range(KH):
            idx = h * KH + k
            ce[:, idx * 128:(idx + 1) * 128] = c[h, k] * eye128
    ce = ce.astype(bf)

    HstackT = np.ascontiguousarray(
        Hstack.astype(np.float32).transpose(0, 2, 1))  # [KH, N, N]
    Hu8 = np.clip(np.rint(HstackT * 255.0), 0, 255).astype(np.uint8)
    wqkvT = np.ascontiguousarray(Wqkv.T).astype(np.float32).copy()
    wqkvT[:, :DIM] *= SCALE                            # fold q scaling
    wpack = np.concatenate(
        [wqkvT, np.ascontiguousarray(Wproj.T)], axis=1).astype(bf)  # [512, 2048]

    in_maps = []
    for i in range(NCORES):
        pk = np.zeros((PACK_ROWS, 1024), bf)
        xi = x[i * BPC:(i + 1) * BPC].reshape(TOK, DIM)
        pk[ROW_X:ROW_X + 1024] = np.ascontiguousarray(xi.T).astype(bf).reshape(1024, 1024)
        pk[ROW_W:ROW_W + 128] = wpack[i * 64:(i + 1) * 64].reshape(128, 1024)
        pk[ROW_CE:ROW_CE + 80] = ce[i * 16:(i + 1) * 16].reshape(80, 1024)
        pk[ROW_MISC, 0:128] = 1.0
        pk[ROW_MISC, 128:640] = bproj.astype(bf)
        hsh = Hu8[:, i * 128:(i + 1) * 128, :].reshape(KH * 128, 1024)
        in_maps.append(dict(pack=pk, hu8=np.ascontiguousarray(hsh)))
    return in_maps


def kernel(**inputs):
    from concourse.bass_utils import run_bass_kernel_spmd

    if "nc" not in _CACHE:
        _CACHE["nc"] = _build()
    nc = _CACHE["nc"]
    in_maps = _prep_host(
        np.asarray(inputs["x"], np.float32),
        np.asarray(inputs["Hstack"], np.float32),
        np.asarray(inputs["hop_logits_attn"], np.float32),
        np.asarray(inputs["rel_alpha"], np.float32),
        np.asarray(inputs["Wqkv"], np.float32),
        np.asarray(inputs["Wproj"], np.float32),
        np.asarray(inputs["bproj"], np.float32))
    res = run_bass_kernel_spmd(nc, in_maps, list(range(NCORES))).results
    out = np.concatenate(
        [r["y"].astype(np.float32).reshape(BPC, N, DIM) for r in res], axis=0)
    return out


# revision 28
# speedup vs baseline: 4.4902x; 1.0534x over previous
"""Trainium2 Bass kernel for nn_Attention_xxc (dense transformer attention
with hop-distance bias). Data-parallel over batch: 8 cores x 2 batches.

Transfer/dispatch-optimized v4. The metric is warm wall time of
run_bass_kernel_spmd; measured cost structure on this axon-tunneled path:
  - host->device upload ~13.4 ns/B, download ~13.4 ns/B (outputs also pay
    a zeros-donation upload), per-call pjit recompile ~0.4s unless the jax
    persistent compilation cache is on, device DMA ~0.23 ms/instruction +
    ~2.6 ns/B, collective (RDH) bytes ~free, device compute ~free.
Hence:
  - ONE packed bf16 input [1249, 1024] (~2.6MB) + u8 Hstack shard
    [640, 1024] (~0.65MB) per core instead of 20.3MB replicated tensors:
    xT + m-row shard of HstackT (u8-quantized, x255) + 1/8 of
    [Wqkv.T|Wproj.T] + 1/8 of the scaled-identity mixture blocks (with
    the 1/255 dequant folded in) + eye + ones/bproj row.
  - Device AllGathers rebuild weights and mixture blocks; each core
    computes biasT rows [128c, 128c+128) for ALL heads (its HstackT
    shard), and one AllGather assembles the full 16MB biasT.
  - DMAs are batched: one instruction per logical load/store group
    (weights, x, H, bias-write, per-head bias load, per-batch y store).
  - Output y in fp16 (download + zeros upload are the costliest bytes).
  - jax persistent compilation cache kills the per-call recompile.

Per-core compute layout (as the proven v1): q,k transposed bf16; v natural
with a ones column per head (denominator); S.T = k.q + bias.T via
identity-matmul PSUM accumulation; exp on ACT; AV -> out.T with row 64 =
denom; normalize via reciprocal broadcast; proj + bproj -> y.
"""
import sys

sys.path.insert(0, "/opt/trn_rl_repo")

import numpy as np
import ml_dtypes
import jax

# Persistent compilation cache: run_bass_kernel_spmd re-lowers and
# re-compiles its jit wrapper on every call (fresh closure -> jit cache
# miss); with the disk cache the per-call XLA/neuronx compile becomes a
# ~137KB cache hit instead of ~0.4s of recompilation.
jax.config.update("jax_compilation_cache_dir", "/tmp/jaxcache")
jax.config.update("jax_persistent_cache_min_compile_time_secs", 0)
jax.config.update("jax_persistent_cache_min_entry_size_bytes", 0)

B, N, DIM = 16, 1024, 512
H, HD, KH = 8, 64, 5
SCALE = HD ** -0.5
NCORES = 8
BPC = B // NCORES          # batches per core
TOK = BPC * N              # tokens per core = 2048

# packed bf16 input row offsets (1024-wide rows)
ROW_W = 0        # 128 rows: weight shard [64, 2048]
ROW_CE = 128     # 80 rows: c_eye shard [16, 5120]
ROW_MISC = 208   # 1 row: ones [0:128], bproj [128:640]
PACK_ROWS = 209
# xT ships as separate u8 byte planes (hi plane compresses ~2x on the
# tunnel since bf16 exponents of N(0,1) data cluster): xsplit[0:512] = hi
# bytes of xT [512, 2048], xsplit[512:1024] = lo bytes.

_CACHE = {}


def _build():
    import concourse.bass as bass
    import concourse.bacc as bacc
    import concourse.mybir as mybir
    from concourse.tile import TileContext

    f32 = mybir.dt.float32
    bf16 = mybir.dt.bfloat16
    f16 = mybir.dt.float16
    u8 = mybir.dt.uint8
    EXP = mybir.ActivationFunctionType.Exp
    MUL = mybir.AluOpType.mult
    ADD = mybir.AluOpType.add
    BYP = mybir.AluOpType.bypass

    u16 = mybir.dt.uint16
    nc = bacc.Bacc(num_devices=NCORES)
    pack = nc.declare_dram_parameter("pack", [PACK_ROWS, 1024], bf16, isOutput=False)
    hu8 = nc.declare_dram_parameter("hu8", [KH * 128, 1024], u8, isOutput=False)
    xsplit = nc.declare_dram_parameter("xsplit", [1024, TOK], u8, isOutput=False)
    y = nc.declare_dram_parameter("y", [TOK, DIM], f16, isOutput=True)

    NT = TOK // 128            # 16 token tiles
    VW = H * (HD + 1)          # 520: v row width with ones col per head
    RG = [list(range(NCORES))]

    with TileContext(nc) as tc:
        with (
            tc.tile_pool(name="dram", bufs=1, space="DRAM") as DR,
            tc.tile_pool(name="const", bufs=1) as CONST,
            tc.tile_pool(name="qk", bufs=1) as QK,
            tc.tile_pool(name="vres", bufs=1) as VR,
            tc.tile_pool(name="outT", bufs=1) as OT,
        ):
            # ---- DRAM bounces, AllGathers (overlap with qkv phase) ----
            wb = DR.tile([128, 1024], bf16, tag="wb", name="wb")
            agw = DR.tile([1024, 1024], bf16, tag="agw", name="agw")
            cb = DR.tile([80, 1024], bf16, tag="cb", name="cb")
            agc = DR.tile([640, 1024], bf16, tag="agc", name="agc")
            bb = DR.tile([H * 128, 1024], bf16, tag="bb", name="bb")
            btd = DR.tile([NCORES, H, 128, 1024], bf16, tag="btd", name="btd")

            nc.sync.dma_start(out=wb[:], in_=pack[ROW_W:ROW_W + 128, :])
            nc.sync.dma_start(out=cb[:], in_=pack[ROW_CE:ROW_CE + 80, :])
            nc.gpsimd.collective_compute(
                "AllGather", BYP, RG, [wb[:].opt()], [agw[:].opt()])
            nc.gpsimd.collective_compute(
                "AllGather", BYP, RG, [cb[:].opt()], [agc[:].opt()])

            # ---- constants (batched loads) ----
            misc_t = CONST.tile([1, 640], bf16, tag="misc", name="misc")
            nc.sync.dma_start(out=misc_t[:], in_=pack[ROW_MISC:ROW_MISC + 1, 0:640])
            ones_t = misc_t[:, 0:128]
            bpr_t = misc_t[:, 128:640]
            ce_t = CONST.tile([128, H * KH * 128], bf16, tag="ce", name="ce")
            nc.sync.dma_start(
                out=ce_t[:], in_=agc[:].rearrange("(p r) c -> p (r c)", r=5))
            # all weights in one DMA: wall[:, c*2048:(c+1)*2048] holds
            # wqkvT rows [128c,128c+128) (cols 0:1536) | wprojT (cols 1536:)
            wall = CONST.tile([128, 8192], bf16, tag="wall", name="wall")
            nc.sync.dma_start(
                out=wall[:].rearrange("p (c r n) -> p c r n", c=4, r=2),
                in_=agw[:].rearrange("(c p r) n -> p c r n", c=4, r=2))
            wq = [wall[:, c * 2048:c * 2048 + 1536] for c in range(4)]
            wp = [wall[:, c * 2048 + 1536:(c + 1) * 2048] for c in range(4)]

            qk_t = [QK.tile([128, TOK], bf16, tag=f"qk{o}", name=f"qk{o}")
                    for o in range(8)]
            v_t = [VR.tile([128, VW], bf16, tag=f"v{t}", name=f"v{t}")
                   for t in range(NT)]
            oT_t = [OT.tile([128, N], bf16, tag=f"oT{b}_{c}", name=f"oT{b}_{c}")
                    for b in range(BPC) for c in range(4)]
            bpb_t = CONST.tile([128, DIM], f32, tag="bpb", name="bpb")

            # ---------------- phase 1: qkv projections ----------------
            with (
                tc.tile_pool(name="xw", bufs=1) as XW,
                tc.tile_pool(name="ps1", bufs=4, space="PSUM") as PS1,
            ):
                # bproj broadcast to 128 partitions via outer product
                psb0 = PS1.tile([128, DIM], f32, tag="ps1", name="ps1")
                nc.tensor.matmul(psb0[:], ones_t, bpr_t,
                                 start=True, stop=True)
                nc.vector.tensor_copy(bpb_t[:], psb0[:])

                # xT arrives as two u8 byte planes; reassemble to bf16 via
                # u16 arithmetic (hi*256 + lo) and bitcast. Bit-exact vs
                # shipping bf16 directly; the hi plane compresses on the
                # tunnel. Layout: xall[:, c*2048 + col] = xT[128c+p, col].
                xhi = XW.tile([128, 8192], u8, tag="xhi", name="xhi")
                xlo = XW.tile([128, 8192], u8, tag="xlo", name="xlo")
                nc.sync.dma_start(
                    out=xhi[:].rearrange("p (c n) -> p c n", c=4),
                    in_=xsplit[0:512, :].rearrange("(c p) n -> p c n", p=128))
                nc.sync.dma_start(
                    out=xlo[:].rearrange("p (c n) -> p c n", c=4),
                    in_=xsplit[512:1024, :].rearrange("(c p) n -> p c n", p=128))
                xall = XW.tile([128, 8192], u16, tag="xall", name="xall")
                nc.vector.tensor_scalar(xall[:], xhi[:], 256, None,
                                        mybir.AluOpType.mult)
                nc.vector.tensor_tensor(xall[:], xall[:], xlo[:], ADD)
                xT = [xall[:, c * 2048:(c + 1) * 2048].bitcast(bf16)
                      for c in range(4)]

                # q,k transposed: qkvT[o_tile, tok] ; o tiles 0..7 cover q,k
                for o in range(8):
                    for t in range(4):           # tok chunks of 512
                        ps = PS1.tile([128, 512], f32, tag="ps1", name="ps1")
                        for c in range(4):
                            nc.tensor.matmul(
                                ps[:], wq[c][:, o * 128:(o + 1) * 128],
                                xT[c][:, t * 512:(t + 1) * 512],
                                start=(c == 0), stop=(c == 3))
                        nc.vector.tensor_copy(qk_t[o][:, t * 512:(t + 1) * 512], ps[:])
                # v natural: [tok_tile, vch] -> packed per head with ones col
                for t in range(NT):
                    ps = PS1.tile([128, 512], f32, tag="ps1", name="ps1")
                    for c in range(4):
                        nc.tensor.matmul(
                            ps[:], xT[c][:, t * 128:(t + 1) * 128],
                            wq[c][:, 2 * DIM:3 * DIM],
                            start=(c == 0), stop=(c == 3))
                    dst = v_t[t][:, 0:VW].rearrange("p (h s) -> p h s", s=HD + 1)
                    nc.vector.tensor_copy(
                        dst[:, :, 0:HD],
                        ps[:].rearrange("p (h s) -> p h s", s=HD))
                    nc.vector.memset(dst[:, :, HD:HD + 1], 1.0)

            # ------- phase 1.5: m-row-sharded bias mixture + AllGather -----
            # This core holds u8 HstackT_k[128c:128c+128, :]*255 for all k;
            # it computes biasT[h][those rows] for ALL heads via scaled-eye
            # stationaries ce_t (1/255 dequant folded in), then one
            # AllGather assembles the full biasT:
            # btd[c, h, p, :] = biasT[h][128c + p, :].
            with (
                tc.tile_pool(name="hbt", bufs=1) as HBT,
                tc.tile_pool(name="bw", bufs=1) as BW,
                tc.tile_pool(name="psb", bufs=2, space="PSUM") as PSB,
            ):
                hu = HBT.tile([128, KH * 1024], u8, tag="hu", name="hu")
                nc.sync.dma_start(
                    out=hu[:].rearrange("p (k n) -> p k n", k=KH),
                    in_=hu8[:].rearrange("(k p) n -> p k n", p=128))
                hall = HBT.tile([128, KH * 1024], bf16, tag="hall", name="hall")
                nc.vector.tensor_copy(hall[:], hu[:])
                ball = BW.tile([128, H * 1024], bf16, tag="ball", name="ball")
                for h in range(H):
                    psb = PSB.tile([128, N], f32, tag="psb", name="psb")
                    for chunk in range(2):
                        sl = slice(chunk * 512, (chunk + 1) * 512)
                        for k in range(KH):
                            idx = h * KH + k
                            nc.tensor.matmul(
                                psb[:, sl],
                                ce_t[:, idx * 128:(idx + 1) * 128],
                                hall[:, k * 1024:k * 1024 + 1024][:, sl],
                                start=(k == 0), stop=(k == KH - 1))
                    # store exp(bias): scores use P = exp(S) * exp(bias)
                    nc.scalar.activation(
                        ball[:, h * 1024:(h + 1) * 1024], psb[:], EXP)
                nc.sync.dma_start(
                    out=bb[:].rearrange("(h p) n -> p h n", p=128),
                    in_=ball[:].rearrange("p (h n) -> p h n", h=H))
                nc.gpsimd.collective_compute(
                    "AllGather", BYP, RG, [bb[:].opt()], [btd[:].opt()])

            # ---------------- phase 2: attention ----------------
            with (
                tc.tile_pool(name="biasp", bufs=2) as BP,
                tc.tile_pool(name="pp", bufs=14) as PP,
                tc.tile_pool(name="nrm", bufs=4) as NRM,
                tc.tile_pool(name="ysb", bufs=2) as YSB,
                tc.tile_pool(name="pss", bufs=2, space="PSUM") as PSS,
                tc.tile_pool(name="pso", bufs=1, space="PSUM") as PSO,
                tc.tile_pool(name="psm", bufs=2, space="PSUM") as PSM,
            ):
                for h in range(H):
                    qt, po = qk_t[h // 2], (h % 2) * 64
                    kt = qk_t[4 + h // 2]
                    # all 8 exp-bias m-tiles for this head in one DMA
                    b_all = BP.tile([128, 8 * 1024], bf16, tag="bias", name="bias")
                    nc.sync.dma_start(
                        out=b_all[:].rearrange("p (m c) -> p m c", m=8),
                        in_=btd[:, h, :, :].rearrange("m p c -> p m c"))
                    for b in range(BPC):
                        t0 = b * N
                        p_tiles = []
                        for mi in range(8):
                            ps = PSS.tile([128, N], f32, tag="pss", name="pss")
                            for nchunk in range(2):
                                sl = slice(nchunk * 512, (nchunk + 1) * 512)
                                nc.tensor.matmul(
                                    ps[:, sl],
                                    kt[po:po + 64, t0 + mi * 128: t0 + (mi + 1) * 128],
                                    qt[po:po + 64, t0 + nchunk * 512: t0 + (nchunk + 1) * 512],
                                    start=True, stop=True)
                            pt = PP.tile([128, N], bf16, tag="p", name="p")
                            nc.scalar.activation(pt[:], ps[:], EXP)
                            nc.vector.tensor_tensor(
                                pt[:], pt[:],
                                b_all[:, mi * 1024:(mi + 1) * 1024], MUL)
                            p_tiles.append(pt)
                        pso = PSO.tile([HD + 1, N], f32, tag="pso", name="pso")
                        for mi in range(8):
                            for nchunk in range(2):
                                sl = slice(nchunk * 512, (nchunk + 1) * 512)
                                nc.tensor.matmul(
                                    pso[:, sl],
                                    v_t[b * 8 + mi][:, h * (HD + 1):(h + 1) * (HD + 1)],
                                    p_tiles[mi][:, sl],
                                    start=(mi == 0), stop=(mi == 7))
                        # denominator -> broadcast -> reciprocal -> normalize
                        d_t = NRM.tile([1, N], bf16, tag="d", name="d")
                        nc.vector.tensor_copy(d_t[:], pso[64:65, :])
                        R_t = NRM.tile([64, N], f32, tag="R", name="R")
                        for nchunk in range(2):
                            sl = slice(nchunk * 512, (nchunk + 1) * 512)
                            psr = PSM.tile([64, 512], f32, tag="psm", name="psm")
                            nc.tensor.matmul(psr[:], ones_t[:, 0:64], d_t[:, sl],
                                             start=True, stop=True)
                            nc.vector.reciprocal(R_t[:, sl], psr[:])
                        nc.vector.tensor_tensor(
                            oT_t[b * 4 + h // 2][po:po + 64, :],
                            pso[0:64, :], R_t[:], MUL)
                # ---------------- phase 3: output projection ----------------
                for b in range(BPC):
                    yt = YSB.tile([128, 8 * DIM], f16, tag="y", name="y")
                    for t in range(8):
                        psy = PSM.tile([128, 512], f32, tag="psm", name="psm")
                        for c in range(4):
                            nc.tensor.matmul(
                                psy[:],
                                oT_t[b * 4 + c][:, t * 128:(t + 1) * 128],
                                wp[c], start=(c == 0), stop=(c == 3))
                        nc.vector.tensor_tensor(
                            yt[:, t * DIM:(t + 1) * DIM], psy[:], bpb_t[:], ADD)
                    nc.sync.dma_start(
                        out=y[b * N:(b + 1) * N, :].rearrange(
                            "(t p) c -> p t c", p=128),
                        in_=yt[:].rearrange("p (t c) -> p t c", t=8))
    nc.compile()
    return nc


def _prep_host(x, Hstack, hop_logits_attn, rel_alpha, Wqkv, Wproj, bproj):
    bf = ml_dtypes.bfloat16
    lg = hop_logits_attn - hop_logits_attn.max(-1, keepdims=True)
    w = np.exp(lg)
    w /= w.sum(-1, keepdims=True)                      # [H, KH]
    # 1/255 dequant for the u8 Hstack folded into the mixture scales
    c = (rel_alpha[:, None] * w).astype(np.float32) / 255.0   # [H, KH]
    # scaled-identity blocks [128, H*KH*128]
    ce = np.zeros((128, H * KH * 128), np.float32)
    eye128 = np.eye(128, dtype=np.float32)
    for h in range(H):
        for k in range(KH):
            idx = h * KH + k
            ce[:, idx * 128:(idx + 1) * 128] = c[h, k] * eye128
    ce = ce.astype(bf)

    HstackT = np.ascontiguousarray(
        Hstack.astype(np.float32).transpose(0, 2, 1))  # [KH, N, N]
    Hu8 = np.clip(np.rint(HstackT * 255.0), 0, 255).astype(np.uint8)
    wqkvT = np.ascontiguousarray(Wqkv.T).astype(np.float32).copy()
    wqkvT[:, :DIM] *= SCALE                            # fold q scaling
    wpack = np.concatenate(
        [wqkvT, np.ascontiguousarray(Wproj.T)], axis=1).astype(bf)  # [512, 2048]

    in_maps = []
    for i in range(NCORES):
        pk = np.zeros((PACK_ROWS, 1024), bf)
        xi = x[i * BPC:(i + 1) * BPC].reshape(TOK, DIM)
        xT_u8 = np.ascontiguousarray(xi.T).astype(bf).view(np.uint8)  # [512, 4096]
        xsp = np.empty((1024, TOK), np.uint8)
        xsp[0:512] = xT_u8[:, 1::2]   # hi bytes (little-endian)
        xsp[512:1024] = xT_u8[:, 0::2]  # lo bytes
        pk[ROW_W:ROW_W + 128] = wpack[i * 64:(i + 1) * 64].reshape(128, 1024)
        pk[ROW_CE:ROW_CE + 80] = ce[i * 16:(i + 1) * 16].reshape(80, 1024)
        pk[ROW_MISC, 0:128] = 1.0
        pk[ROW_MISC, 128:640] = bproj.astype(bf)
        hsh = Hu8[:, i * 128:(i + 1) * 128, :].reshape(KH * 128, 1024)
        in_maps.append(dict(pack=pk, hu8=np.ascontiguousarray(hsh), xsplit=xsp))
    return in_maps


def kernel(**inputs):
    from concourse.bass_utils import run_bass_kernel_spmd

    if "nc" not in _CACHE:
        _CACHE["nc"] = _build()
    nc = _CACHE["nc"]
    in_maps = _prep_host(
        np.asarray(inputs["x"], np.float32),
        np.asarray(inputs["Hstack"], np.float32),
        np.asarray(inputs["hop_logits_attn"], np.float32),
        np.asarray(inputs["rel_alpha"], np.float32),
        np.asarray(inputs["Wqkv"], np.float32),
        np.asarray(inputs["Wproj"], np.float32),
        np.asarray(inputs["bproj"], np.float32))
    res = run_bass_kernel_spmd(nc, in_maps, list(range(NCORES))).results
    out = np.concatenate(
        [r["y"].astype(np.float32).reshape(BPC, N, DIM) for r in res], axis=0)
    return out


# revision 32
# speedup vs baseline: 4.5946x; 1.0233x over previous
"""Trainium2 Bass kernel for nn_Attention_xxc (dense transformer attention
with hop-distance bias). Data-parallel over batch: 8 cores x 2 batches.

Transfer/dispatch-optimized v4. The metric is warm wall time of
run_bass_kernel_spmd; measured cost structure on this axon-tunneled path:
  - host->device upload ~13.4 ns/B, download ~13.4 ns/B (outputs also pay
    a zeros-donation upload), per-call pjit recompile ~0.4s unless the jax
    persistent compilation cache is on, device DMA ~0.23 ms/instruction +
    ~2.6 ns/B, collective (RDH) bytes ~free, device compute ~free.
Hence:
  - ONE packed bf16 input [1249, 1024] (~2.6MB) + u8 Hstack shard
    [640, 1024] (~0.65MB) per core instead of 20.3MB replicated tensors:
    xT + m-row shard of HstackT (u8-quantized, x255) + 1/8 of
    [Wqkv.T|Wproj.T] + 1/8 of the scaled-identity mixture blocks (with
    the 1/255 dequant folded in) + eye + ones/bproj row.
  - Device AllGathers rebuild weights and mixture blocks; each core
    computes biasT rows [128c, 128c+128) for ALL heads (its HstackT
    shard), and one AllGather assembles the full 16MB biasT.
  - DMAs are batched: one instruction per logical load/store group
    (weights, x, H, bias-write, per-head bias load, per-batch y store).
  - Output y in fp16 (download + zeros upload are the costliest bytes).
  - jax persistent compilation cache kills the per-call recompile.

Per-core compute layout (as the proven v1): q,k transposed bf16; v natural
with a ones column per head (denominator); S.T = k.q + bias.T via
identity-matmul PSUM accumulation; exp on ACT; AV -> out.T with row 64 =
denom; normalize via reciprocal broadcast; proj + bproj -> y.
"""
import sys

sys.path.insert(0, "/opt/trn_rl_repo")

import numpy as np
import ml_dtypes
import jax

# Persistent compilation cache: run_bass_kernel_spmd re-lowers and
# re-compiles its jit wrapper on every call (fresh closure -> jit cache
# miss); with the disk cache the per-call XLA/neuronx compile becomes a
# ~137KB cache hit instead of ~0.4s of recompilation.
jax.config.update("jax_compilation_cache_dir", "/tmp/jaxcache")
jax.config.update("jax_persistent_cache_min_compile_time_secs", 0)
jax.config.update("jax_persistent_cache_min_entry_size_bytes", 0)

B, N, DIM = 16, 1024, 512
H, HD, KH = 8, 64, 5
SCALE = HD ** -0.5
NCORES = 8
BPC = B // NCORES          # batches per core
TOK = BPC * N              # tokens per core = 2048

# packed bf16 input row offsets (1024-wide rows)
ROW_W = 0        # 128 rows: weight shard [64, 2048]
ROW_CE = 128     # 80 rows: c_eye shard [16, 5120]
ROW_MISC = 208   # 1 row: ones [0:128], bproj [128:640]
PACK_ROWS = 209
# xT ships as separate u8 byte planes (hi plane compresses ~2x on the
# tunnel since bf16 exponents of N(0,1) data cluster): xsplit[0:512] = hi
# bytes of xT [512, 2048], xsplit[512:1024] = lo bytes.

_CACHE = {}


def _build():
    import concourse.bass as bass
    import concourse.bacc as bacc
    import concourse.mybir as mybir
    from concourse.tile import TileContext

    f32 = mybir.dt.float32
    bf16 = mybir.dt.bfloat16
    f16 = mybir.dt.float16
    u8 = mybir.dt.uint8
    EXP = mybir.ActivationFunctionType.Exp
    MUL = mybir.AluOpType.mult
    ADD = mybir.AluOpType.add
    BYP = mybir.AluOpType.bypass

    u16 = mybir.dt.uint16
    nc = bacc.Bacc(num_devices=NCORES)
    pack = nc.declare_dram_parameter("pack", [PACK_ROWS, 1024], bf16, isOutput=False)
    hu8 = nc.declare_dram_parameter("hu8", [KH * 128, 1024], u8, isOutput=False)
    xsplit = nc.declare_dram_parameter("xsplit", [1024, TOK], u8, isOutput=False)
    y = nc.declare_dram_parameter("y", [TOK, DIM], f16, isOutput=True)

    NT = TOK // 128            # 16 token tiles
    VW = H * (HD + 1)          # 520: v row width with ones col per head
    RG = [list(range(NCORES))]

    with TileContext(nc) as tc:
        with (
            tc.tile_pool(name="dram", bufs=1, space="DRAM") as DR,
            tc.tile_pool(name="const", bufs=1) as CONST,
            tc.tile_pool(name="qk", bufs=1) as QK,
            tc.tile_pool(name="vres", bufs=1) as VR,
            tc.tile_pool(name="outT", bufs=1) as OT,
        ):
            # ---- DRAM bounces, AllGathers (overlap with qkv phase) ----
            wb = DR.tile([128, 1024], bf16, tag="wb", name="wb")
            agw = DR.tile([1024, 1024], bf16, tag="agw", name="agw")
            cb = DR.tile([80, 1024], bf16, tag="cb", name="cb")
            agc = DR.tile([640, 1024], bf16, tag="agc", name="agc")
            bb = DR.tile([H * 128, 1024], bf16, tag="bb", name="bb")
            btd = DR.tile([NCORES, H, 128, 1024], bf16, tag="btd", name="btd")

            nc.sync.dma_start(out=wb[:], in_=pack[ROW_W:ROW_W + 128, :])
            nc.sync.dma_start(out=cb[:], in_=pack[ROW_CE:ROW_CE + 80, :])
            nc.gpsimd.collective_compute(
                "AllGather", BYP, RG, [wb[:].opt()], [agw[:].opt()])
            nc.gpsimd.collective_compute(
                "AllGather", BYP, RG, [cb[:].opt()], [agc[:].opt()])

            # ---- constants (batched loads) ----
            misc_t = CONST.tile([1, 640], bf16, tag="misc", name="misc")
            nc.sync.dma_start(out=misc_t[:], in_=pack[ROW_MISC:ROW_MISC + 1, 0:640])
            ones_t = misc_t[:, 0:128]
            bpr_t = misc_t[:, 128:640]
            ce_t = CONST.tile([128, H * KH * 128], bf16, tag="ce", name="ce")
            nc.sync.dma_start(
                out=ce_t[:], in_=agc[:].rearrange("(p r) c -> p (r c)", r=5))
            # all weights in one DMA: wall[:, c*2048:(c+1)*2048] holds
            # wqkvT rows [128c,128c+128) (cols 0:1536) | wprojT (cols 1536:)
            wall = CONST.tile([128, 8192], bf16, tag="wall", name="wall")
            nc.sync.dma_start(
                out=wall[:].rearrange("p (c r n) -> p c r n", c=4, r=2),
                in_=agw[:].rearrange("(c p r) n -> p c r n", c=4, r=2))
            wq = [wall[:, c * 2048:c * 2048 + 1536] for c in range(4)]
            wp = [wall[:, c * 2048 + 1536:(c + 1) * 2048] for c in range(4)]

            qk_t = [QK.tile([128, TOK], bf16, tag=f"qk{o}", name=f"qk{o}")
                    for o in range(8)]
            v_t = [VR.tile([128, VW], bf16, tag=f"v{t}", name=f"v{t}")
                   for t in range(NT)]
            oT_t = [OT.tile([128, N], bf16, tag=f"oT{b}_{c}", name=f"oT{b}_{c}")
                    for b in range(BPC) for c in range(4)]
            bpb_t = CONST.tile([128, DIM], f32, tag="bpb", name="bpb")

            # ---------------- phase 1: qkv projections ----------------
            with (
                tc.tile_pool(name="xw", bufs=1) as XW,
                tc.tile_pool(name="ps1", bufs=4, space="PSUM") as PS1,
            ):
                # bproj broadcast to 128 partitions via outer product
                psb0 = PS1.tile([128, DIM], f32, tag="ps1", name="ps1")
                nc.tensor.matmul(psb0[:], ones_t, bpr_t,
                                 start=True, stop=True)
                nc.vector.tensor_copy(bpb_t[:], psb0[:])

                # xT arrives as two u8 byte planes; reassemble to bf16 via
                # u16 arithmetic (hi*256 + lo) and bitcast. Bit-exact vs
                # shipping bf16 directly; the hi plane compresses on the
                # tunnel. Layout: xall[:, c*2048 + col] = xT[128c+p, col].
                xhi = XW.tile([128, 8192], u8, tag="xhi", name="xhi")
                xlo = XW.tile([128, 8192], u8, tag="xlo", name="xlo")
                nc.sync.dma_start(
                    out=xhi[:].rearrange("p (c n) -> p c n", c=4),
                    in_=xsplit[0:512, :].rearrange("(c p) n -> p c n", p=128))
                nc.sync.dma_start(
                    out=xlo[:].rearrange("p (c n) -> p c n", c=4),
                    in_=xsplit[512:1024, :].rearrange("(c p) n -> p c n", p=128))
                xall = XW.tile([128, 8192], u16, tag="xall", name="xall")
                nc.vector.tensor_scalar(xall[:], xhi[:], 256, None,
                                        mybir.AluOpType.mult)
                nc.vector.tensor_tensor(xall[:], xall[:], xlo[:], ADD)
                xT = [xall[:, c * 2048:(c + 1) * 2048].bitcast(bf16)
                      for c in range(4)]

                # q,k transposed: qkvT[o_tile, tok] ; o tiles 0..7 cover q,k
                for o in range(8):
                    for t in range(4):           # tok chunks of 512
                        ps = PS1.tile([128, 512], f32, tag="ps1", name="ps1")
                        for c in range(4):
                            nc.tensor.matmul(
                                ps[:], wq[c][:, o * 128:(o + 1) * 128],
                                xT[c][:, t * 512:(t + 1) * 512],
                                start=(c == 0), stop=(c == 3))
                        nc.vector.tensor_copy(qk_t[o][:, t * 512:(t + 1) * 512], ps[:])
                # v natural: [tok_tile, vch] -> packed per head with ones col
                for t in range(NT):
                    ps = PS1.tile([128, 512], f32, tag="ps1", name="ps1")
                    for c in range(4):
                        nc.tensor.matmul(
                            ps[:], xT[c][:, t * 128:(t + 1) * 128],
                            wq[c][:, 2 * DIM:3 * DIM],
                            start=(c == 0), stop=(c == 3))
                    dst = v_t[t][:, 0:VW].rearrange("p (h s) -> p h s", s=HD + 1)
                    nc.vector.tensor_copy(
                        dst[:, :, 0:HD],
                        ps[:].rearrange("p (h s) -> p h s", s=HD))
                    nc.vector.memset(dst[:, :, HD:HD + 1], 1.0)

            # ------- phase 1.5: m-row-sharded bias mixture + AllGather -----
            # This core holds u8 HstackT_k[128c:128c+128, :]*255 for all k;
            # it computes biasT[h][those rows] for ALL heads via scaled-eye
            # stationaries ce_t (1/255 dequant folded in), then one
            # AllGather assembles the full biasT:
            # btd[c, h, p, :] = biasT[h][128c + p, :].
            with (
                tc.tile_pool(name="hbt", bufs=1) as HBT,
                tc.tile_pool(name="bw", bufs=1) as BW,
                tc.tile_pool(name="psb", bufs=2, space="PSUM") as PSB,
            ):
                hu = HBT.tile([128, KH * 1024], u8, tag="hu", name="hu")
                nc.sync.dma_start(
                    out=hu[:].rearrange("p (k n) -> p k n", k=KH),
                    in_=hu8[:].rearrange("(k p) n -> p k n", p=128))
                hall = HBT.tile([128, KH * 1024], bf16, tag="hall", name="hall")
                nc.vector.tensor_copy(hall[:], hu[:])
                ball = BW.tile([128, H * 1024], bf16, tag="ball", name="ball")
                for h in range(H):
                    psb = PSB.tile([128, N], f32, tag="psb", name="psb")
                    for chunk in range(2):
                        sl = slice(chunk * 512, (chunk + 1) * 512)
                        for k in range(KH):
                            idx = h * KH + k
                            nc.tensor.matmul(
                                psb[:, sl],
                                ce_t[:, idx * 128:(idx + 1) * 128],
                                hall[:, k * 1024:k * 1024 + 1024][:, sl],
                                start=(k == 0), stop=(k == KH - 1))
                    # store exp(bias): scores use P = exp(S) * exp(bias)
                    nc.scalar.activation(
                        ball[:, h * 1024:(h + 1) * 1024], psb[:], EXP)
                nc.sync.dma_start(
                    out=bb[:].rearrange("(h p) n -> p h n", p=128),
                    in_=ball[:].rearrange("p (h n) -> p h n", h=H))
                nc.gpsimd.collective_compute(
                    "AllGather", BYP, RG, [bb[:].opt()], [btd[:].opt()])

            # ---------------- phase 2: attention ----------------
            with (
                tc.tile_pool(name="biasp", bufs=2) as BP,
                tc.tile_pool(name="pp", bufs=14) as PP,
                tc.tile_pool(name="nrm", bufs=4) as NRM,
                tc.tile_pool(name="ysb", bufs=2) as YSB,
                tc.tile_pool(name="pss", bufs=2, space="PSUM") as PSS,
                tc.tile_pool(name="pso", bufs=1, space="PSUM") as PSO,
                tc.tile_pool(name="psm", bufs=2, space="PSUM") as PSM,
            ):
                for h in range(H):
                    qt, po = qk_t[h // 2], (h % 2) * 64
                    kt = qk_t[4 + h // 2]
                    # all 8 exp-bias m-tiles for this head in one DMA
                    b_all = BP.tile([128, 8 * 1024], bf16, tag="bias", name="bias")
                    nc.sync.dma_start(
                        out=b_all[:].rearrange("p (m c) -> p m c", m=8),
                        in_=btd[:, h, :, :].rearrange("m p c -> p m c"))
                    for b in range(BPC):
                        t0 = b * N
                        p_tiles = []
                        for mi in range(8):
                            ps = PSS.tile([128, N], f32, tag="pss", name="pss")
                            for nchunk in range(2):
                                sl = slice(nchunk * 512, (nchunk + 1) * 512)
                                nc.tensor.matmul(
                                    ps[:, sl],
                                    kt[po:po + 64, t0 + mi * 128: t0 + (mi + 1) * 128],
                                    qt[po:po + 64, t0 + nchunk * 512: t0 + (nchunk + 1) * 512],
                                    start=True, stop=True)
                            pt = PP.tile([128, N], bf16, tag="p", name="p")
                            nc.scalar.activation(pt[:], ps[:], EXP)
                            nc.vector.tensor_tensor(
                                pt[:], pt[:],
                                b_all[:, mi * 1024:(mi + 1) * 1024], MUL)
                            p_tiles.append(pt)
                        pso = PSO.tile([HD + 1, N], f32, tag="pso", name="pso")
                        for mi in range(8):
                            for nchunk in range(2):
                                sl = slice(nchunk * 512, (nchunk + 1) * 512)
                                nc.tensor.matmul(
                                    pso[:, sl],
                                    v_t[b * 8 + mi][:, h * (HD + 1):(h + 1) * (HD + 1)],
                                    p_tiles[mi][:, sl],
                                    start=(mi == 0), stop=(mi == 7))
                        # denominator -> broadcast -> reciprocal -> normalize
                        d_t = NRM.tile([1, N], bf16, tag="d", name="d")
                        nc.vector.tensor_copy(d_t[:], pso[64:65, :])
                        R_t = NRM.tile([64, N], f32, tag="R", name="R")
                        for nchunk in range(2):
                            sl = slice(nchunk * 512, (nchunk + 1) * 512)
                            psr = PSM.tile([64, 512], f32, tag="psm", name="psm")
                            nc.tensor.matmul(psr[:], ones_t[:, 0:64], d_t[:, sl],
                                             start=True, stop=True)
                            nc.vector.reciprocal(R_t[:, sl], psr[:])
                        nc.vector.tensor_tensor(
                            oT_t[b * 4 + h // 2][po:po + 64, :],
                            pso[0:64, :], R_t[:], MUL)
                # ---------------- phase 3: output projection ----------------
                for b in range(BPC):
                    yt = YSB.tile([128, 8 * DIM], f16, tag="y", name="y")
                    for t in range(8):
                        psy = PSM.tile([128, 512], f32, tag="psm", name="psm")
                        for c in range(4):
                            nc.tensor.matmul(
                                psy[:],
                                oT_t[b * 4 + c][:, t * 128:(t + 1) * 128],
                                wp[c], start=(c == 0), stop=(c == 3))
                        nc.vector.tensor_tensor(
                            yt[:, t * DIM:(t + 1) * DIM], psy[:], bpb_t[:], ADD)
                    nc.sync.dma_start(
                        out=y[b * N:(b + 1) * N, :].rearrange(
                            "(t p) c -> p t c", p=128),
                        in_=yt[:].rearrange("p (t c) -> p t c", t=8))
    nc.compile()
    return nc


def _prep_host(x, Hstack, hop_logits_attn, rel_alpha, Wqkv, Wproj, bproj):
    bf = ml_dtypes.bfloat16
    lg = hop_logits_attn - hop_logits_attn.max(-1, keepdims=True)
    w = np.exp(lg)
    w /= w.sum(-1, keepdims=True)                      # [H, KH]
    # 1/255 dequant for the u8 Hstack folded into the mixture scales
    c = (rel_alpha[:, None] * w).astype(np.float32) / 255.0   # [H, KH]
    # scaled-identity blocks [128, H*KH*128]
    ce = np.zeros((128, H * KH * 128), np.float32)
    eye128 = np.eye(128, dtype=np.float32)
    for h in range(H):
        for k in range(KH):
            idx = h * KH + k
            ce[:, idx * 128:(idx + 1) * 128] = c[h, k] * eye128
    ce = ce.astype(bf)

    HstackT = np.ascontiguousarray(
        Hstack.astype(np.float32).transpose(0, 2, 1))  # [KH, N, N]
    Hu8 = np.clip(np.rint(HstackT * 255.0), 0, 255).astype(np.uint8)
    wqkvT = np.ascontiguousarray(Wqkv.T).astype(np.float32).copy()
    wqkvT[:, :DIM] *= SCALE                            # fold q scaling
    wpack = np.concatenate(
        [wqkvT, np.ascontiguousarray(Wproj.T)], axis=1).astype(bf)  # [512, 2048]

    in_maps = []
    for i in range(NCORES):
        pk = np.zeros((PACK_ROWS, 1024), bf)
        xi = x[i * BPC:(i + 1) * BPC].reshape(TOK, DIM)
        xT_u8 = np.ascontiguousarray(xi.T).astype(bf).view(np.uint8)  # [512, 4096]
        xsp = np.empty((1024, TOK), np.uint8)
        xsp[0:512] = xT_u8[:, 1::2]   # hi bytes (little-endian)
        xsp[512:1024] = xT_u8[:, 0::2]  # lo bytes
        pk[ROW_W:ROW_W + 128] = wpack[i * 64:(i + 1) * 64].reshape(128, 1024)
        pk[ROW_CE:ROW_CE + 80] = ce[i * 16:(i + 1) * 16].reshape(80, 1024)
        pk[ROW_MISC, 0:128] = 1.0
        pk[ROW_MISC, 128:640] = bproj.astype(bf)
        hsh = Hu8[:, i * 128:(i + 1) * 128, :].reshape(KH * 128, 1024)
        in_maps.append(dict(pack=pk, hu8=np.ascontiguousarray(hsh), xsplit=xsp))
    return in_maps


def kernel(**inputs):
    from concourse.bass_utils import run_bass_kernel_spmd

    if "nc" not in _CACHE:
        _CACHE["nc"] = _build()
    nc = _CACHE["nc"]
    in_maps = _prep_host(
        np.asarray(inputs["x"], np.float32),
        np.asarray(inputs["Hstack"], np.float32),
        np.asarray(inputs["hop_logits_attn"], np.float32),
        np.asarray(inputs["rel_alpha"], np.float32),
        np.asarray(inputs["Wqkv"], np.float32),
        np.asarray(inputs["Wproj"], np.float32),
        np.asarray(inputs["bproj"], np.float32))
    res = run_bass_kernel_spmd(nc, in_maps, list(range(NCORES))).results
    out = np.concatenate(
        [r["y"].astype(np.float32).reshape(BPC, N, DIM) for r in res], axis=0)
    return out
